# revision 31
# baseline (speedup 1.0000x reference)
"""Trainium2 Bass kernel for the EvoSA block (depthwise-conv positional
encoding + attention with global_attn stats + MLP).

Self-contained: takes FULL inputs as in reference.setup_inputs(), shards
batch B=16 across 8 NeuronCores (2 per core), returns FULL outputs
(cls_out, x_out, global_attn).

Layout strategy (per core, 2 batch elements):
  - activations channel-major [C, N] (channels on partitions, tokens free)
  - LN affine folded into the following matmul weights on host
  - depthwise conv as 9 diagonal-weight matmuls accumulating in PSUM
  - attention scores computed transposed S^T[m,n] = k_m.q_n so softmax's
    denominator comes from a ones-column appended to V in the attn@v matmul
  - exp without max-subtraction (scores are ~N(0,0.15); overflow impossible)
"""
import sys

sys.path.insert(0, '/opt/trn_rl_repo')

import numpy as np
import ml_dtypes

import concourse.bass as bass
import concourse.tile as tile
from concourse import mybir
from concourse.vector_clock import ScopedClock, VectorClock

BF16 = ml_dtypes.bfloat16

# ---------------------------------------------------------------------------
# Environment patches.
# (1) walrus on this build accepts only one sync-wait per CTRL instruction:
#     split the TileContext tail-drain into single-wait drains.
# (2) the trimmed repo lacks antenv.axon_hooks; recreate it so
#     run_bass_kernel_spmd(trace=True) can profile via NTFF.
# ---------------------------------------------------------------------------


def _drain_and_barrier_split(self, tick_clock, wait_clock):
    gc = list(tick_clock.global_clock)
    nonzero = [i for i, t in enumerate(gc) if t > 0]
    for i in nonzero:
        sub = [gc[j] if j == i else 0 for j in range(len(gc))]
        drain_inst = self.nc.sync.drain()
        wait_clock.add_sem_waits(drain_inst.ins,
                                 ScopedClock({None: VectorClock(sub)}))
    if not nonzero:
        self.nc.sync.drain()
    self.nc.all_engine_barrier()
    assert self.sems is not None
    popped = self.nc._tile_sem_poison_stack.pop()
    assert popped is self._sem_poison
    self.nc.clear_and_free_semaphores(list(self.sems.allocated().values()))
    self.nc.all_engine_barrier()


tile.TileContext._drain_and_barrier = _drain_and_barrier_split


def _split_excess_waits(nc, max_waits=1):
    """This walrus build accepts only one sync-wait command per instruction.
    Move excess waits onto preceding same-engine NOPs."""
    import bass_rust
    nsplit = 0
    for bb in nc.main_func.blocks:
        out = []
        changed = False
        for ins in bb.instructions:
            si = ins.sync_info
            waits = list(si.on_wait) if si is not None and si.on_wait else []
            # DMA waits are queue-level (descriptor) waits; moving them to a
            # sequencer NOP would stall the whole queue-push stream and can
            # deadlock. Leave them alone.

            if len(waits) > max_waits:
                extra, keep = waits[:-max_waits], waits[-max_waits:]
                for k, i0 in enumerate(range(0, len(extra), max_waits)):
                    nop = mybir.InstNoOp(name=f"{ins.name}-ws{k}", ins=[],
                                         outs=[])
                    nop.engine = ins.engine
                    nop.sync_info = bass_rust.SyncInfo(
                        on_wait=extra[i0:i0 + max_waits], on_update=[])
                    out.append(nop)
                    nsplit += 1
                ins.sync_info = bass_rust.SyncInfo(
                    on_wait=keep, on_update=list(si.on_update))
                changed = True
            out.append(ins)
        if changed:
            bb.instructions = out
    return nsplit


def _install_ntff_hook():
    import types
    try:
        import antenv
        if hasattr(antenv, 'axon_hooks'):
            return
        mod = types.ModuleType('antenv.axon_hooks')
        _h = [None]
        mod.set_axon_ntff_profile_hook = lambda h: _h.__setitem__(0, h)
        mod.get_axon_ntff_profile_hook = lambda: _h[0]
        sys.modules['antenv.axon_hooks'] = mod
        antenv.axon_hooks = mod
        from trn_agent_boot.trn_boot import _ntff_profile_via_ctypes
        mod.set_axon_ntff_profile_hook(
            _ntff_profile_via_ctypes('/opt/axon/libaxon_pjrt.so'))
    except Exception:
        pass


_install_ntff_hook()

# ---------------------------------------------------------------------------
# Problem constants (hardcoded per contract)
# ---------------------------------------------------------------------------
B, C, H, W = 16, 384, 28, 28
HEADS = 8
D = C // HEADS                   # 48
N = H * W + 1                    # 785 tokens (cls + 784 patches)
HWP = H * W                      # 784
NCORES = 8
BPC = B // NCORES                # 2 batches per core
CC = C // 128                    # 3 channel chunks
QP = 64                          # per-head padded width in q/k layout
QROWS = HEADS * QP               # 512
HID = 4 * C                      # 1536
HC = HID // 128                  # 12
SCALE = float(D) ** -0.5
PADW = 30                        # padded spatial width (30x30)

NSPLIT = [(0, 512), (512, N - 512)]          # matmul N<=512 splits of 785
TOKC = [(i * 128, min(128, N - i * 128)) for i in range((N + 127) // 128)]

F32 = mybir.dt.float32
BF = mybir.dt.bfloat16
ADD = mybir.AluOpType.add
MULT = mybir.AluOpType.mult

TRACE = False          # set by test harness for profiled runs
LAST_EXEC_NS = None


def _pbcast(nc, dpool, src, dst, which):
    """Broadcast src [1, F] to dst [P, F] across partitions by bouncing
    through DRAM (DRAM source APs allow a zero partition step; SBUF ones
    don't, and this walrus build can't codegen gpsimd partition_broadcast)."""
    f = src.shape[-1]
    dtmp = dpool.tile([1, f], src.dtype, name=f"dtmp{which}", tag="dtmp",
                      bufs=4)
    nc.gpsimd.dma_start(out=dtmp, in_=src)
    nc.gpsimd.dma_start(out=dst, in_=bass.AP(
        tensor=dtmp.tensor, offset=dtmp.offset,
        ap=[[0, dst.shape[0]]] + [list(dtmp.ap[-1])]))


def _layernorm(nc, pools, wts, xc, which):
    """y = (xc - mu) * rstd in bf16, channel-major. LN affine is folded into
    the consuming matmul weights on the host."""
    wpool, abuf, inbuf, scratch, ppool, pspool, dpool = pools
    s1 = pspool.tile([1, N], F32, name=f"s1{which}", tag="ps")
    s2 = pspool.tile([1, N], F32, name=f"s2{which}", tag="ps")
    for c in range(CC):
        xb = scratch.tile([128, N], BF, name=f"xb{which}{c}", tag="scratch_bf")
        nc.gpsimd.tensor_copy(out=xb, in_=xc[c])
        sq = scratch.tile([128, N], BF, name=f"sq{which}{c}", tag="scratch_bf")
        nc.gpsimd.tensor_mul(out=sq, in0=xc[c], in1=xc[c])
        for (o, w) in NSPLIT:
            nc.tensor.matmul(s1[:, o:o + w], wts['ones_col'], xb[:, o:o + w],
                             start=(c == 0), stop=(c == CC - 1))
            nc.tensor.matmul(s2[:, o:o + w], wts['ones_col'], sq[:, o:o + w],
                             start=(c == 0), stop=(c == CC - 1))
    # var*C = s2 - s1^2/C ; std = sqrt(var*C/C + eps); rstd = 1/std
    t1 = scratch.tile([1, N], F32, name=f"t1{which}", tag="ln_small")
    nc.scalar.square(out=t1, in_=s1)
    v384 = scratch.tile([1, N], F32, name=f"v384{which}", tag="ln_small")
    nc.vector.scalar_tensor_tensor(out=v384, in0=t1, scalar=-1.0 / C, in1=s2,
                                   op0=MULT, op1=ADD)
    std = scratch.tile([1, N], F32, name=f"std{which}", tag="ln_small")
    nc.scalar.activation(out=std, in_=v384,
                         func=mybir.ActivationFunctionType.Sqrt,
                         bias=1e-5, scale=1.0 / C)
    rstd = scratch.tile([1, N], BF, name=f"rstd{which}", tag="ln_small")
    with nc.allow_low_precision(reason="rstd feeds bf16 y anyway"):
        nc.vector.reciprocal(out=rstd, in_=std)
    nm = scratch.tile([1, N], BF, name=f"nm{which}", tag="ln_small")
    nc.vector.scalar_tensor_tensor(out=nm, in0=s1, scalar=-1.0 / C, in1=rstd,
                                   op0=MULT, op1=MULT)
    # broadcast rstd and nm across partitions via K=1 ones-outer matmuls
    # (keeps the LN critical path on PE; avoids DMA-ring round trips)
    rbp = pspool.tile([128, N], F32, name=f"rbp{which}", tag="ps")
    nbp = pspool.tile([128, N], F32, name=f"nbp{which}", tag="ps")
    for (o, w) in NSPLIT:
        nc.tensor.matmul(rbp[:, o:o + w], wts['ones_row'], rstd[:, o:o + w],
                         start=True, stop=True)
        nc.tensor.matmul(nbp[:, o:o + w], wts['ones_row'], nm[:, o:o + w],
                         start=True, stop=True)
    y = []
    for c in range(CC):
        yt = scratch.tile([128, N], BF, name=f"yt{which}{c}", tag="scratch_bf")
        nc.vector.tensor_mul(out=yt, in0=xc[c], in1=rbp)
        yb = abuf.tile([128, N], BF, name=f"y{which}{c}", tag=f"y{c}", bufs=2)
        nc.vector.tensor_add(out=yb, in0=yt, in1=nbp)
        y.append(yb)
    return y


def _emit_batch(nc, tc, pools, wts, prm, b):
    wpool, abuf, inbuf, scratch, ppool, pspool, dpool = pools

    # ---------------- Phase A: conv + residual + xc^T assembly ------------
    xpad, xf32 = [], []
    for c in range(CC):
        xpad_t = inbuf.tile([128, PADW, PADW], BF, name=f"xpad{c}",
                            tag=f"xpad{c}")
        nc.sync.dma_start(out=xpad_t,
                          in_=prm['xpad_bf'][b, c * 128:(c + 1) * 128]
                          .rearrange("p (h w) -> p h w", h=PADW))
        xpad.append(xpad_t)
        xf_t = inbuf.tile([128, HWP], F32, name=f"xf{c}", tag=f"xf{c}")
        nc.sync.dma_start(out=xf_t, in_=prm['x_f32'][b, c * 128:(c + 1) * 128])
        xf32.append(xf_t)

    xc = []
    for c in range(CC):
        xc_t = abuf.tile([128, N], F32, name=f"xc{c}", tag=f"xc{c}", bufs=2)
        nc.sync.dma_start(out=xc_t[:, 0:1],
                          in_=prm['cls'][b, c * 128:(c + 1) * 128])
        for half in range(2):
            cps = pspool.tile([128, 392], F32, name=f"cps{c}{half}", tag="ps")
            r0 = half * 14
            for t in range(9):
                ti, tj = divmod(t, 3)
                rhs = xpad[c][:, ti + r0: ti + r0 + 14, tj: tj + W]
                nc.tensor.matmul(cps, wts['dconv'][c][:, t, :], rhs,
                                 start=(t == 0), stop=(t == 8))
            nc.vector.scalar_tensor_tensor(
                out=xc_t[:, 1 + half * 392: 1 + (half + 1) * 392], in0=cps,
                scalar=wts['conv_b'][:, c:c + 1],
                in1=xf32[c][:, half * 392:(half + 1) * 392], op0=ADD, op1=ADD)
        xc.append(xc_t)

    # ---------------- Phase B: LN1 -> y ----------------------------------
    y = _layernorm(nc, pools, wts, xc, f"a{b}")

    # ---------------- Phase C: q^T, k^T (head-padded), v (token-major) ---
    qT, kT = [], []
    for name, wkey, bkey, dst in (("q", 'wq', 'bq', qT), ("k", 'wk', 'bk', kT)):
        for j in range(QROWS // 128):
            ps = pspool.tile([128, N], F32, name=f"ps{name}{j}", tag="ps")
            for c in range(CC):
                for (o, w) in NSPLIT:
                    nc.tensor.matmul(ps[:, o:o + w],
                                     wts[wkey][c][:, j * 128:(j + 1) * 128],
                                     y[c][:, o:o + w],
                                     start=(c == 0), stop=(c == CC - 1))
            ot = abuf.tile([128, N], BF, name=f"{name}T{j}", tag=f"{name}T{j}")
            nc.vector.tensor_scalar_add(out=ot, in0=ps,
                                        scalar1=wts[bkey][:, j:j + 1])
            dst.append(ot)

    v_sb = []
    for mc, (ms, mr) in enumerate(TOKC):
        ps = pspool.tile([128, C], F32, name=f"psv{mc}", tag="ps")
        for c in range(CC):
            nc.tensor.matmul(ps[:mr, :], y[c][:, ms:ms + mr], wts['wv'][c],
                             start=(c == 0), stop=(c == CC - 1))
        vt = abuf.tile([128, HEADS, D + 1], BF, name=f"v{mc}", tag=f"v{mc}")
        nc.gpsimd.memset(vt[:mr, :, 0:1], 1.0)
        nc.vector.scalar_tensor_tensor(
            out=vt[:mr, :, 1:D + 1],
            in0=ps[:mr, :].rearrange("p (h d) -> p h d", h=HEADS),
            scalar=1.0,
            in1=wts['bv_bcast'][:mr, :].rearrange("p (h d) -> p h d", h=HEADS),
            op0=MULT, op1=ADD)
        v_sb.append(vt)

    # ---------------- Phase D: attention, head at a time ------------------
    o_bf = []
    r0_row = abuf.tile([1, HEADS], BF, name="r0row", tag="r0row")
    gcols = [abuf.tile([128, HEADS], BF, name=f"gcol{mc}", tag=f"gcol{mc}")
             for mc in range(len(TOKC))]

    for j in range(HEADS // 2):
        he, ho = 2 * j, 2 * j + 1
        ops_e = pspool.tile([D + 1, N], F32, name=f"opse{j}", tag="ps")
        ops_o = pspool.tile([D + 1, N], F32, name=f"opso{j}", tag="ps")
        for mc, (ms, mr) in enumerate(TOKC):
            sps_e = pspool.tile([128, N], F32, name=f"spse{j}{mc}", tag="ps")
            sps_o = pspool.tile([128, N], F32, name=f"spso{j}{mc}", tag="ps")
            for (o, w) in NSPLIT:
                nc.tensor.matmul(sps_e[:mr, o:o + w], kT[j][0:QP, ms:ms + mr],
                                 qT[j][0:QP, o:o + w], start=True, stop=True)
            for (o, w) in NSPLIT:
                nc.tensor.matmul(sps_o[:mr, o:o + w], kT[j][QP:128, ms:ms + mr],
                                 qT[j][QP:128, o:o + w], start=True, stop=True)
            pt_e = ppool.tile([128, N], BF, name=f"pe{j}{mc}", tag="pt")
            nc.scalar.activation(out=pt_e[:mr, :], in_=sps_e[:mr, :],
                                 func=mybir.ActivationFunctionType.Exp,
                                 scale=SCALE)
            pt_o = ppool.tile([128, N], BF, name=f"po{j}{mc}", tag="pt")
            nc.scalar.activation(out=pt_o[:mr, :], in_=sps_o[:mr, :],
                                 func=mybir.ActivationFunctionType.Exp,
                                 scale=SCALE)
            nc.gpsimd.tensor_copy(out=gcols[mc][:mr, he:he + 1],
                                  in_=pt_e[:mr, 0:1])
            nc.gpsimd.tensor_copy(out=gcols[mc][:mr, ho:ho + 1],
                                  in_=pt_o[:mr, 0:1])
            last = mc == len(TOKC) - 1
            for (o, w) in NSPLIT:
                nc.tensor.matmul(ops_e[0:D + 1, o:o + w], v_sb[mc][:mr, he, :],
                                 pt_e[:mr, o:o + w],
                                 start=(mc == 0), stop=last)
            for (o, w) in NSPLIT:
                nc.tensor.matmul(ops_o[0:D + 1, o:o + w], v_sb[mc][:mr, ho, :],
                                 pt_o[:mr, o:o + w],
                                 start=(mc == 0), stop=last)
        for hh, opsx in ((he, ops_e), (ho, ops_o)):
            rec = abuf.tile([1, N], BF, name=f"rec{hh}", tag="rec", bufs=2)
            with nc.allow_low_precision(reason="recip consumed in bf16"):
                nc.vector.reciprocal(out=rec, in_=opsx[0:1, 0:N])
            nc.gpsimd.tensor_copy(out=r0_row[:, hh:hh + 1], in_=rec[:, 0:1])
            rbs = scratch.tile([D + 1, N], BF, name=f"rbs{hh}", tag="rbs",
                               bufs=2)
            _pbcast(nc, pools[-1], rec, rbs, f"rec{hh}")
            ob = abuf.tile([D + 1, N], BF, name=f"ob{hh}", tag=f"ob{hh}")
            nc.vector.tensor_mul(out=ob, in0=opsx[:, 0:N], in1=rbs)
            o_bf.append(ob)

    # ---------------- global_attn ----------------------------------------
    r0b = pspool.tile([128, HEADS], F32, name="r0b", tag="ps")
    nc.tensor.matmul(r0b, wts['ones_row'], r0_row, start=True, stop=True)
    for mc, (ms, mr) in enumerate(TOKC):
        gs = abuf.tile([128, 1], F32, name=f"gs{mc}", tag=f"gs{mc}")
        nc.vector.scalar_tensor_tensor(
            out=gcols[mc][:mr, :], in0=gcols[mc][:mr, :], scalar=1.0 / HEADS,
            in1=r0b[:mr, :], op0=MULT, op1=MULT, accum_out=gs[:mr, :])
        if mc == 0:
            nc.gpsimd.dma_start(out=prm['gattn'][b, 0:mr - 1], in_=gs[1:mr, :])
        else:
            nc.gpsimd.dma_start(out=prm['gattn'][b, ms - 1:ms - 1 + mr],
                                in_=gs[:mr, :])

    # ---------------- Phase E: proj + residual ---------------------------
    for c in range(CC):
        ps = pspool.tile([128, N], F32, name=f"psp{c}", tag="ps")
        for hh in range(HEADS):
            for (o, w) in NSPLIT:
                nc.tensor.matmul(ps[:, o:o + w],
                                 wts['wproj'][hh][:, c * 128:(c + 1) * 128],
                                 o_bf[hh][:, o:o + w],
                                 start=(hh == 0), stop=(hh == HEADS - 1))
        nc.vector.scalar_tensor_tensor(
            out=xc[c], in0=ps, scalar=wts['bproj'][:, c:c + 1],
            in1=xc[c], op0=ADD, op1=ADD)

    # ---------------- Phase F/G: LN2 + MLP -------------------------------
    y2 = _layernorm(nc, pools, wts, xc, f"b{b}")
    h_bf = []
    for j in range(HC):
        ps = pspool.tile([128, N], F32, name=f"psh{j}", tag="ps")
        for c in range(CC):
            for (o, w) in NSPLIT:
                nc.tensor.matmul(ps[:, o:o + w],
                                 wts['wfc1'][c][:, j * 128:(j + 1) * 128],
                                 y2[c][:, o:o + w],
                                 start=(c == 0), stop=(c == CC - 1))
        ht = abuf.tile([128, N], BF, name=f"h{j}", tag=f"h{j}")
        nc.scalar.activation(out=ht, in_=ps,
                             func=mybir.ActivationFunctionType.Gelu,
                             bias=wts['bfc1'][:, j:j + 1], scale=1.0)
        h_bf.append(ht)
    for c in range(CC):
        ps = pspool.tile([128, N], F32, name=f"psf{c}", tag="ps")
        for j in range(HC):
            for (o, w) in NSPLIT:
                nc.tensor.matmul(ps[:, o:o + w],
                                 wts['wfc2'][j][:, c * 128:(c + 1) * 128],
                                 h_bf[j][:, o:o + w],
                                 start=(j == 0), stop=(j == HC - 1))
        nc.vector.scalar_tensor_tensor(
            out=xc[c], in0=ps, scalar=wts['bfc2'][:, c:c + 1],
            in1=xc[c], op0=ADD, op1=ADD)

    # ---------------- Phase H: outputs -----------------------------------
    for c in range(CC):
        nc.gpsimd.dma_start(out=prm['x_out'][b, c * 128:(c + 1) * 128, :],
                            in_=xc[c][:, 1:N])
        nc.gpsimd.dma_start(out=prm['cls_out'][b, c * 128:(c + 1) * 128],
                            in_=xc[c][:, 0:1])


def _register_const(nc, dtype, value):
    t = nc.alloc_sbuf_tensor(f"const-{dtype.name}-{value}", [128, 1], dtype)
    nc.gpsimd.memset(t.ap(), value)
    nc.const_aps.aps[(dtype, value)] = t.ap()


def build_program():
    nc = bass.Bass()
    _register_const(nc, mybir.dt.float32, 1e-5)
    nc.all_engine_barrier()
    prm = {}
    dp = nc.declare_dram_parameter
    prm['xpad_bf'] = dp("xpad_bf", [BPC, C, PADW * PADW], BF, isOutput=False)
    prm['x_f32'] = dp("x_f32", [BPC, C, HWP], F32, isOutput=False)
    prm['cls'] = dp("cls", [BPC, C, 1], F32, isOutput=False)
    prm['dconv'] = dp("dconv", [CC, 128, 9, 128], BF, isOutput=False)
    prm['conv_b'] = dp("conv_b", [128, CC], F32, isOutput=False)
    prm['wq'] = dp("wq", [C, QROWS], BF, isOutput=False)
    prm['wk'] = dp("wk", [C, QROWS], BF, isOutput=False)
    prm['wv'] = dp("wv", [C, C], BF, isOutput=False)
    prm['bq'] = dp("bq", [128, QROWS // 128], F32, isOutput=False)
    prm['bk'] = dp("bk", [128, QROWS // 128], F32, isOutput=False)
    prm['bv_bcast'] = dp("bv_bcast", [128, C], F32, isOutput=False)
    prm['wproj'] = dp("wproj", [HEADS, D + 1, C], BF, isOutput=False)
    prm['bproj'] = dp("bproj", [128, CC], F32, isOutput=False)
    prm['wfc1'] = dp("wfc1", [C, HID], BF, isOutput=False)
    prm['bfc1'] = dp("bfc1", [128, HC], F32, isOutput=False)
    prm['wfc2'] = dp("wfc2", [HID, C], BF, isOutput=False)
    prm['bfc2'] = dp("bfc2", [128, CC], F32, isOutput=False)
    prm['cls_out'] = dp("cls_out", [BPC, C, 1], F32, isOutput=True)
    prm['x_out'] = dp("x_out", [BPC, C, HWP], F32, isOutput=True)
    prm['gattn'] = dp("gattn", [BPC, HWP], F32, isOutput=True)

    with tile.TileContext(nc) as tc:
        import contextlib
        with contextlib.ExitStack() as ctx:
            wpool = ctx.enter_context(tc.tile_pool(name="wpool", bufs=1))
            abuf = ctx.enter_context(tc.tile_pool(name="abuf", bufs=1))
            inbuf = ctx.enter_context(tc.tile_pool(name="inbuf", bufs=2))
            scratch = ctx.enter_context(tc.tile_pool(name="scratch", bufs=3))
            ppool = ctx.enter_context(tc.tile_pool(name="ppool", bufs=3))
            pspool = ctx.enter_context(
                tc.tile_pool(name="pspool", bufs=4, space="PSUM"))
            dpool = ctx.enter_context(
                tc.tile_pool(name="dpool", bufs=1, space="DRAM"))
            pools = (wpool, abuf, inbuf, scratch, ppool, pspool, dpool)

            wts = {}
            dconv = []
            for c in range(CC):
                t = wpool.tile([128, 9, 128], BF, name=f"dconv{c}",
                               tag=f"dconv{c}")
                nc.scalar.dma_start(out=t, in_=prm['dconv'][c])
                dconv.append(t)
            wts['dconv'] = dconv
            for key, rows, width in (('wq', C, QROWS), ('wk', C, QROWS),
                                     ('wv', C, C), ('wfc1', C, HID)):
                ts = []
                for c in range(rows // 128):
                    t = wpool.tile([128, width], BF, name=f"{key}{c}",
                                   tag=f"{key}{c}")
                    nc.scalar.dma_start(out=t,
                                      in_=prm[key][c * 128:(c + 1) * 128])
                    ts.append(t)
                wts[key] = ts
            ts = []
            for j in range(HC):
                t = wpool.tile([128, C], BF, name=f"wfc2{j}", tag=f"wfc2{j}")
                nc.scalar.dma_start(out=t, in_=prm['wfc2'][j * 128:(j + 1) * 128])
                ts.append(t)
            wts['wfc2'] = ts
            ts = []
            for hh in range(HEADS):
                t = wpool.tile([D + 1, C], BF, name=f"wproj{hh}",
                               tag=f"wproj{hh}")
                nc.scalar.dma_start(out=t, in_=prm['wproj'][hh])
                ts.append(t)
            wts['wproj'] = ts
            for key, shape in (('conv_b', [128, CC]), ('bq', [128, 4]),
                               ('bk', [128, 4]), ('bproj', [128, CC]),
                               ('bfc1', [128, HC]), ('bfc2', [128, CC]),
                               ('bv_bcast', [128, C])):
                t = wpool.tile(shape, F32, name=f"w_{key}", tag=f"w_{key}")
                nc.scalar.dma_start(out=t, in_=prm[key][:, :])
                wts[key] = t
            ones_col = wpool.tile([128, 1], BF, name="ones_col",
                                  tag="ones_col")
            nc.vector.memset(ones_col, 1.0)
            wts['ones_col'] = ones_col
            ones_row = wpool.tile([1, 128], BF, name="ones_row",
                                  tag="ones_row")
            nc.vector.memset(ones_row, 1.0)
            wts['ones_row'] = ones_row

            for b in range(BPC):
                _emit_batch(nc, tc, pools, wts, prm, b)
    _split_excess_waits(nc)
    return nc, prm


_CACHED = None


def _get_program():
    global _CACHED
    if _CACHED is None:
        _CACHED = build_program()
    return _CACHED


def _prep_host(inputs):
    """Fold LN affine into weights, transpose/pad into lhsT layouts, pad x
    for SAME conv, cast matmul operands to bf16."""
    f32 = np.float32
    x = np.asarray(inputs['x'], f32)
    cls_token = np.asarray(inputs['cls_token'], f32)
    conv_w = np.asarray(inputs['conv_w'], f32)
    conv_b = np.asarray(inputs['conv_b'], f32)
    ln1_g = np.asarray(inputs['ln1_g'], f32)
    ln1_b = np.asarray(inputs['ln1_b'], f32)
    qkv_w = np.asarray(inputs['qkv_w'], f32)
    proj_w = np.asarray(inputs['proj_w'], f32)
    proj_b = np.asarray(inputs['proj_b'], f32)
    ln2_g = np.asarray(inputs['ln2_g'], f32)
    ln2_b = np.asarray(inputs['ln2_b'], f32)
    fc1_w = np.asarray(inputs['fc1_w'], f32)
    fc1_b = np.asarray(inputs['fc1_b'], f32)
    fc2_w = np.asarray(inputs['fc2_w'], f32)
    fc2_b = np.asarray(inputs['fc2_b'], f32)

    def colmajor(v):          # [C*] vector -> [128, C*/128] chunk-col layout
        return np.ascontiguousarray(v.reshape(-1, 128).T).astype(f32)

    shared = {}
    wt = conv_w[:, 0].reshape(C, 9)
    dconv = np.zeros((CC, 128, 9, 128), f32)
    idx = np.arange(128)
    for c in range(CC):
        dconv[c, idx, :, idx] = wt[c * 128:(c + 1) * 128, :]
    shared['dconv'] = dconv.astype(BF16)
    shared['conv_b'] = colmajor(conv_b)

    qkv_g = qkv_w * ln1_g[None, :]
    qkv_bias = qkv_w @ ln1_b
    wq_f, wk_f, wv_f = qkv_g[0:C], qkv_g[C:2 * C], qkv_g[2 * C:3 * C]
    bq_f, bk_f, bv_f = qkv_bias[0:C], qkv_bias[C:2 * C], qkv_bias[2 * C:3 * C]

    def pad_qk(wf):
        out = np.zeros((C, QROWS), f32)
        for hh in range(HEADS):
            out[:, hh * QP: hh * QP + D] = wf[hh * D:(hh + 1) * D, :].T
        return out

    def pad_qk_bias(bf_):
        out = np.zeros((QROWS,), f32)
        for hh in range(HEADS):
            out[hh * QP: hh * QP + D] = bf_[hh * D:(hh + 1) * D]
        return colmajor(out)

    shared['wq'] = pad_qk(wq_f).astype(BF16)
    shared['wk'] = pad_qk(wk_f).astype(BF16)
    shared['bq'] = pad_qk_bias(bq_f)
    shared['bk'] = pad_qk_bias(bk_f)
    shared['wv'] = wv_f.T.copy().astype(BF16)
    shared['bv_bcast'] = np.tile(bv_f[None, :], (128, 1)).astype(f32)
    wproj = np.zeros((HEADS, D + 1, C), f32)
    wproj[:, 1:, :] = proj_w.T.reshape(HEADS, D, C)
    shared['wproj'] = wproj.astype(BF16)
    shared['bproj'] = colmajor(proj_b)
    shared['wfc1'] = (fc1_w * ln2_g[None, :]).T.copy().astype(BF16)
    shared['bfc1'] = colmajor(fc1_b + fc1_w @ ln2_b)
    shared['wfc2'] = fc2_w.T.copy().astype(BF16)
    shared['bfc2'] = colmajor(fc2_b)

    xf = x.reshape(B, C, HWP)
    xpad = np.zeros((B, C, PADW, PADW), f32)
    xpad[:, :, 1:1 + H, 1:1 + W] = x
    xpad_bf = xpad.reshape(B, C, PADW * PADW).astype(BF16)
    clsr = np.ascontiguousarray(cls_token[:, 0, :]).reshape(B, C, 1)

    in_maps = []
    for core in range(NCORES):
        sl = slice(core * BPC, (core + 1) * BPC)
        m = dict(shared)
        m['xpad_bf'] = np.ascontiguousarray(xpad_bf[sl])
        m['x_f32'] = np.ascontiguousarray(xf[sl])
        m['cls'] = np.ascontiguousarray(clsr[sl])
        in_maps.append(m)
    return in_maps


def kernel(**inputs):
    global LAST_EXEC_NS
    from concourse.bass_utils import run_bass_kernel_spmd
    nc, prm = _get_program()
    in_maps = _prep_host(inputs)
    res = run_bass_kernel_spmd(nc, in_maps, core_ids=list(range(NCORES)),
                               trace=TRACE)
    LAST_EXEC_NS = res.exec_time_ns
    cls_out = np.zeros((B, 1, C), np.float32)
    x_out = np.zeros((B, C, H, W), np.float32)
    gattn = np.zeros((B, HWP), np.float32)
    for core in range(NCORES):
        r = res.results[core]
        sl = slice(core * BPC, (core + 1) * BPC)
        cls_out[sl, 0, :] = r['cls_out'][:, :, 0]
        x_out[sl] = r['x_out'].reshape(BPC, C, H, W)
        gattn[sl] = r['gattn']
    return cls_out, x_out, gattn


# revision 32
# speedup vs baseline: 1.1294x; 1.1294x over previous
"""Trainium2 Bass kernel for the EvoSA block (depthwise-conv positional
encoding + attention with global_attn stats + MLP).

Self-contained: takes FULL inputs as in reference.setup_inputs(), shards
batch B=16 across 8 NeuronCores (2 per core), returns FULL outputs
(cls_out, x_out, global_attn).

Layout strategy (per core, 2 batch elements):
  - activations channel-major [C, N] (channels on partitions, tokens free)
  - LN affine folded into the following matmul weights on host
  - depthwise conv as 9 diagonal-weight matmuls accumulating in PSUM
  - attention scores computed transposed S^T[m,n] = k_m.q_n so softmax's
    denominator comes from a ones-column appended to V in the attn@v matmul
  - exp without max-subtraction (scores are ~N(0,0.15); overflow impossible)
"""
import sys

sys.path.insert(0, '/opt/trn_rl_repo')

import numpy as np
import ml_dtypes

import concourse.bass as bass
import concourse.tile as tile
from concourse import mybir
from concourse.vector_clock import ScopedClock, VectorClock

BF16 = ml_dtypes.bfloat16

# ---------------------------------------------------------------------------
# Environment patches.
# (1) walrus on this build accepts only one sync-wait per CTRL instruction:
#     split the TileContext tail-drain into single-wait drains.
# (2) the trimmed repo lacks antenv.axon_hooks; recreate it so
#     run_bass_kernel_spmd(trace=True) can profile via NTFF.
# ---------------------------------------------------------------------------


def _drain_and_barrier_split(self, tick_clock, wait_clock):
    gc = list(tick_clock.global_clock)
    nonzero = [i for i, t in enumerate(gc) if t > 0]
    for i in nonzero:
        sub = [gc[j] if j == i else 0 for j in range(len(gc))]
        drain_inst = self.nc.sync.drain()
        wait_clock.add_sem_waits(drain_inst.ins,
                                 ScopedClock({None: VectorClock(sub)}))
    if not nonzero:
        self.nc.sync.drain()
    self.nc.all_engine_barrier()
    assert self.sems is not None
    popped = self.nc._tile_sem_poison_stack.pop()
    assert popped is self._sem_poison
    self.nc.clear_and_free_semaphores(list(self.sems.allocated().values()))
    self.nc.all_engine_barrier()


tile.TileContext._drain_and_barrier = _drain_and_barrier_split


def _split_excess_waits(nc, max_waits=1):
    """This walrus build accepts only one sync-wait command per instruction.
    Move excess waits onto preceding same-engine NOPs."""
    import bass_rust
    nsplit = 0
    for bb in nc.main_func.blocks:
        out = []
        changed = False
        for ins in bb.instructions:
            si = ins.sync_info
            waits = list(si.on_wait) if si is not None and si.on_wait else []
            # DMA waits are queue-level (descriptor) waits; moving them to a
            # sequencer NOP would stall the whole queue-push stream and can
            # deadlock. Leave them alone.

            if len(waits) > max_waits:
                extra, keep = waits[:-max_waits], waits[-max_waits:]
                for k, i0 in enumerate(range(0, len(extra), max_waits)):
                    nop = mybir.InstNoOp(name=f"{ins.name}-ws{k}", ins=[],
                                         outs=[])
                    nop.engine = ins.engine
                    nop.sync_info = bass_rust.SyncInfo(
                        on_wait=extra[i0:i0 + max_waits], on_update=[])
                    out.append(nop)
                    nsplit += 1
                ins.sync_info = bass_rust.SyncInfo(
                    on_wait=keep, on_update=list(si.on_update))
                changed = True
            out.append(ins)
        if changed:
            bb.instructions = out
    return nsplit


def _install_ntff_hook():
    import types
    try:
        import antenv
        if hasattr(antenv, 'axon_hooks'):
            return
        mod = types.ModuleType('antenv.axon_hooks')
        _h = [None]
        mod.set_axon_ntff_profile_hook = lambda h: _h.__setitem__(0, h)
        mod.get_axon_ntff_profile_hook = lambda: _h[0]
        sys.modules['antenv.axon_hooks'] = mod
        antenv.axon_hooks = mod
        from trn_agent_boot.trn_boot import _ntff_profile_via_ctypes
        mod.set_axon_ntff_profile_hook(
            _ntff_profile_via_ctypes('/opt/axon/libaxon_pjrt.so'))
    except Exception:
        pass


_install_ntff_hook()

# ---------------------------------------------------------------------------
# Problem constants (hardcoded per contract)
# ---------------------------------------------------------------------------
B, C, H, W = 16, 384, 28, 28
HEADS = 8
D = C // HEADS                   # 48
N = H * W + 1                    # 785 tokens (cls + 784 patches)
HWP = H * W                      # 784
NCORES = 8
BPC = B // NCORES                # 2 batches per core
CC = C // 128                    # 3 channel chunks
QP = 64                          # per-head padded width in q/k layout
QROWS = HEADS * QP               # 512
HID = 4 * C                      # 1536
HC = HID // 128                  # 12
SCALE = float(D) ** -0.5
PADW = 30                        # padded spatial width (30x30)

NSPLIT = [(0, 512), (512, N - 512)]          # matmul N<=512 splits of 785
TOKC = [(i * 128, min(128, N - i * 128)) for i in range((N + 127) // 128)]

F32 = mybir.dt.float32
BF = mybir.dt.bfloat16
ADD = mybir.AluOpType.add
MULT = mybir.AluOpType.mult

TRACE = False          # set by test harness for profiled runs
LAST_EXEC_NS = None


def _pbcast(nc, dpool, src, dst, which):
    """Broadcast src [1, F] to dst [P, F] across partitions by bouncing
    through DRAM (DRAM source APs allow a zero partition step; SBUF ones
    don't, and this walrus build can't codegen gpsimd partition_broadcast)."""
    f = src.shape[-1]
    dtmp = dpool.tile([1, f], src.dtype, name=f"dtmp{which}", tag="dtmp",
                      bufs=4)
    nc.gpsimd.dma_start(out=dtmp, in_=src)
    nc.gpsimd.dma_start(out=dst, in_=bass.AP(
        tensor=dtmp.tensor, offset=dtmp.offset,
        ap=[[0, dst.shape[0]]] + [list(dtmp.ap[-1])]))


def _layernorm(nc, pools, wts, xc, which):
    """y = (xc - mu) * rstd in bf16, channel-major. LN affine is folded into
    the consuming matmul weights on the host."""
    wpool, abuf, inbuf, scratch, ppool, pspool, dpool = pools
    s1 = pspool.tile([1, N], F32, name=f"s1{which}", tag="ps")
    s2 = pspool.tile([1, N], F32, name=f"s2{which}", tag="ps")
    for c in range(CC):
        xb = scratch.tile([128, N], BF, name=f"xb{which}{c}", tag="scratch_bf")
        nc.gpsimd.tensor_copy(out=xb, in_=xc[c])
        sq = scratch.tile([128, N], BF, name=f"sq{which}{c}", tag="scratch_bf")
        nc.gpsimd.tensor_mul(out=sq, in0=xc[c], in1=xc[c])
        for (o, w) in NSPLIT:
            nc.tensor.matmul(s1[:, o:o + w], wts['ones_col'], xb[:, o:o + w],
                             start=(c == 0), stop=(c == CC - 1))
            nc.tensor.matmul(s2[:, o:o + w], wts['ones_col'], sq[:, o:o + w],
                             start=(c == 0), stop=(c == CC - 1))
    # var*C = s2 - s1^2/C ; std = sqrt(var*C/C + eps); rstd = 1/std
    t1 = scratch.tile([1, N], F32, name=f"t1{which}", tag="ln_small")
    nc.scalar.square(out=t1, in_=s1)
    v384 = scratch.tile([1, N], F32, name=f"v384{which}", tag="ln_small")
    nc.vector.scalar_tensor_tensor(out=v384, in0=t1, scalar=-1.0 / C, in1=s2,
                                   op0=MULT, op1=ADD)
    std = scratch.tile([1, N], F32, name=f"std{which}", tag="ln_small")
    nc.scalar.activation(out=std, in_=v384,
                         func=mybir.ActivationFunctionType.Sqrt,
                         bias=1e-5, scale=1.0 / C)
    rstd = scratch.tile([1, N], BF, name=f"rstd{which}", tag="ln_small")
    with nc.allow_low_precision(reason="rstd feeds bf16 y anyway"):
        nc.vector.reciprocal(out=rstd, in_=std)
    nm = scratch.tile([1, N], BF, name=f"nm{which}", tag="ln_small")
    nc.vector.scalar_tensor_tensor(out=nm, in0=s1, scalar=-1.0 / C, in1=rstd,
                                   op0=MULT, op1=MULT)
    # broadcast rstd and nm across partitions via K=1 ones-outer matmuls
    # (keeps the LN critical path on PE; avoids DMA-ring round trips)
    rbp = pspool.tile([128, N], F32, name=f"rbp{which}", tag="ps")
    nbp = pspool.tile([128, N], F32, name=f"nbp{which}", tag="ps")
    for (o, w) in NSPLIT:
        nc.tensor.matmul(rbp[:, o:o + w], wts['ones_row'], rstd[:, o:o + w],
                         start=True, stop=True)
        nc.tensor.matmul(nbp[:, o:o + w], wts['ones_row'], nm[:, o:o + w],
                         start=True, stop=True)
    y = []
    for c in range(CC):
        yt = scratch.tile([128, N], BF, name=f"yt{which}{c}", tag="scratch_bf")
        nc.vector.tensor_mul(out=yt, in0=xc[c], in1=rbp)
        yb = abuf.tile([128, N], BF, name=f"y{which}{c}", tag=f"y{c}", bufs=2)
        nc.vector.tensor_add(out=yb, in0=yt, in1=nbp)
        y.append(yb)
    return y


def _emit_batch(nc, tc, pools, wts, prm, b):
    wpool, abuf, inbuf, scratch, ppool, pspool, dpool = pools

    # ---------------- Phase A: conv + residual + xc^T assembly ------------
    xpad, xf32 = [], []
    for c in range(CC):
        xpad_t = inbuf.tile([128, PADW, PADW], BF, name=f"xpad{c}",
                            tag=f"xpad{c}")
        nc.sync.dma_start(out=xpad_t,
                          in_=prm['xpad_bf'][b, c * 128:(c + 1) * 128]
                          .rearrange("p (h w) -> p h w", h=PADW))
        xpad.append(xpad_t)
        xf_t = inbuf.tile([128, HWP], F32, name=f"xf{c}", tag=f"xf{c}")
        nc.sync.dma_start(out=xf_t, in_=prm['x_f32'][b, c * 128:(c + 1) * 128])
        xf32.append(xf_t)

    xc = []
    for c in range(CC):
        xc_t = abuf.tile([128, N], F32, name=f"xc{c}", tag=f"xc{c}", bufs=2)
        nc.sync.dma_start(out=xc_t[:, 0:1],
                          in_=prm['cls'][b, c * 128:(c + 1) * 128])
        for half in range(2):
            cps = pspool.tile([128, 392], F32, name=f"cps{c}{half}", tag="ps")
            r0 = half * 14
            for t in range(9):
                ti, tj = divmod(t, 3)
                rhs = xpad[c][:, ti + r0: ti + r0 + 14, tj: tj + W]
                nc.tensor.matmul(cps, wts['dconv'][c][:, t, :], rhs,
                                 start=(t == 0), stop=(t == 8))
            nc.vector.scalar_tensor_tensor(
                out=xc_t[:, 1 + half * 392: 1 + (half + 1) * 392], in0=cps,
                scalar=wts['conv_b'][:, c:c + 1],
                in1=xf32[c][:, half * 392:(half + 1) * 392], op0=ADD, op1=ADD)
        xc.append(xc_t)

    # ---------------- Phase B: LN1 -> y ----------------------------------
    y = _layernorm(nc, pools, wts, xc, f"a{b}")

    # ---------------- Phase C: q^T, k^T (head-padded), v (token-major) ---
    qT, kT = [], []
    for name, wkey, bkey, dst in (("q", 'wq', 'bq', qT), ("k", 'wk', 'bk', kT)):
        for j in range(QROWS // 128):
            ps = pspool.tile([128, N], F32, name=f"ps{name}{j}", tag="ps")
            for c in range(CC):
                for (o, w) in NSPLIT:
                    nc.tensor.matmul(ps[:, o:o + w],
                                     wts[wkey][c][:, j * 128:(j + 1) * 128],
                                     y[c][:, o:o + w],
                                     start=(c == 0), stop=(c == CC - 1))
            ot = abuf.tile([128, N], BF, name=f"{name}T{j}", tag=f"{name}T{j}")
            nc.vector.tensor_scalar_add(out=ot, in0=ps,
                                        scalar1=wts[bkey][:, j:j + 1])
            dst.append(ot)

    v_sb = []
    for mc, (ms, mr) in enumerate(TOKC):
        ps = pspool.tile([128, C], F32, name=f"psv{mc}", tag="ps")
        for c in range(CC):
            nc.tensor.matmul(ps[:mr, :], y[c][:, ms:ms + mr], wts['wv'][c],
                             start=(c == 0), stop=(c == CC - 1))
        vt = abuf.tile([128, HEADS, D + 1], BF, name=f"v{mc}", tag=f"v{mc}")
        nc.gpsimd.memset(vt[:mr, :, 0:1], 1.0)
        nc.vector.scalar_tensor_tensor(
            out=vt[:mr, :, 1:D + 1],
            in0=ps[:mr, :].rearrange("p (h d) -> p h d", h=HEADS),
            scalar=1.0,
            in1=wts['bv_bcast'][:mr, :].rearrange("p (h d) -> p h d", h=HEADS),
            op0=MULT, op1=ADD)
        v_sb.append(vt)

    # ---------------- Phase D: attention, head at a time ------------------
    o_bf = []
    r0_row = abuf.tile([1, HEADS], BF, name="r0row", tag="r0row")
    gcols = [abuf.tile([128, HEADS], BF, name=f"gcol{mc}", tag=f"gcol{mc}")
             for mc in range(len(TOKC))]

    for h in range(HEADS):
        jt, jr = h // 2, QP * (h % 2)
        ops = pspool.tile([D + 1, N], F32, name=f"ops{h}", tag="ps")
        for mc, (ms, mr) in enumerate(TOKC):
            sps = pspool.tile([128, N], F32, name=f"sps{h}{mc}", tag="ps")
            for (o, w) in NSPLIT:
                nc.tensor.matmul(sps[:mr, o:o + w],
                                 kT[jt][jr:jr + QP, ms:ms + mr],
                                 qT[jt][jr:jr + QP, o:o + w],
                                 start=True, stop=True)
            pt = ppool.tile([128, N], BF, name=f"p{h}{mc}", tag="pt")
            nc.scalar.activation(out=pt[:mr, :], in_=sps[:mr, :],
                                 func=mybir.ActivationFunctionType.Exp,
                                 scale=SCALE)
            nc.gpsimd.tensor_copy(out=gcols[mc][:mr, h:h + 1],
                                  in_=pt[:mr, 0:1])
            for (o, w) in NSPLIT:
                nc.tensor.matmul(ops[:, o:o + w], v_sb[mc][:mr, h, :],
                                 pt[:mr, o:o + w],
                                 start=(mc == 0), stop=(mc == len(TOKC) - 1))
        rec = abuf.tile([1, N], BF, name=f"rec{h}", tag="rec", bufs=2)
        with nc.allow_low_precision(reason="softmax recip consumed in bf16"):
            nc.vector.reciprocal(out=rec, in_=ops[0:1, :])
        nc.gpsimd.tensor_copy(out=r0_row[:, h:h + 1], in_=rec[:, 0:1])
        rbs = scratch.tile([D + 1, N], BF, name=f"rbs{h}", tag="rbs", bufs=2)
        _pbcast(nc, pools[-1], rec, rbs, f"rec{h}")
        ob = abuf.tile([D + 1, N], BF, name=f"ob{h}", tag=f"ob{h}")
        nc.vector.tensor_mul(out=ob, in0=ops, in1=rbs)
        o_bf.append(ob)

    # ---------------- global_attn ----------------------------------------
    r0b = pspool.tile([128, HEADS], F32, name="r0b", tag="ps")
    nc.tensor.matmul(r0b, wts['ones_row'], r0_row, start=True, stop=True)
    for mc, (ms, mr) in enumerate(TOKC):
        gs = abuf.tile([128, 1], F32, name=f"gs{mc}", tag=f"gs{mc}")
        nc.vector.scalar_tensor_tensor(
            out=gcols[mc][:mr, :], in0=gcols[mc][:mr, :], scalar=1.0 / HEADS,
            in1=r0b[:mr, :], op0=MULT, op1=MULT, accum_out=gs[:mr, :])
        if mc == 0:
            nc.gpsimd.dma_start(out=prm['gattn'][b, 0:mr - 1], in_=gs[1:mr, :])
        else:
            nc.gpsimd.dma_start(out=prm['gattn'][b, ms - 1:ms - 1 + mr],
                                in_=gs[:mr, :])

    # ---------------- Phase E: proj + residual ---------------------------
    for c in range(CC):
        ps = pspool.tile([128, N], F32, name=f"psp{c}", tag="ps")
        for hh in range(HEADS):
            for (o, w) in NSPLIT:
                nc.tensor.matmul(ps[:, o:o + w],
                                 wts['wproj'][hh][:, c * 128:(c + 1) * 128],
                                 o_bf[hh][:, o:o + w],
                                 start=(hh == 0), stop=(hh == HEADS - 1))
        nc.vector.scalar_tensor_tensor(
            out=xc[c], in0=ps, scalar=wts['bproj'][:, c:c + 1],
            in1=xc[c], op0=ADD, op1=ADD)

    # ---------------- Phase F/G: LN2 + MLP -------------------------------
    y2 = _layernorm(nc, pools, wts, xc, f"b{b}")
    h_bf = []
    for j in range(HC):
        ps = pspool.tile([128, N], F32, name=f"psh{j}", tag="ps")
        for c in range(CC):
            for (o, w) in NSPLIT:
                nc.tensor.matmul(ps[:, o:o + w],
                                 wts['wfc1'][c][:, j * 128:(j + 1) * 128],
                                 y2[c][:, o:o + w],
                                 start=(c == 0), stop=(c == CC - 1))
        ht = abuf.tile([128, N], BF, name=f"h{j}", tag=f"h{j}")
        nc.scalar.activation(out=ht, in_=ps,
                             func=mybir.ActivationFunctionType.Gelu,
                             bias=wts['bfc1'][:, j:j + 1], scale=1.0)
        h_bf.append(ht)
    for c in range(CC):
        ps = pspool.tile([128, N], F32, name=f"psf{c}", tag="ps")
        for j in range(HC):
            for (o, w) in NSPLIT:
                nc.tensor.matmul(ps[:, o:o + w],
                                 wts['wfc2'][j][:, c * 128:(c + 1) * 128],
                                 h_bf[j][:, o:o + w],
                                 start=(j == 0), stop=(j == HC - 1))
        nc.vector.scalar_tensor_tensor(
            out=xc[c], in0=ps, scalar=wts['bfc2'][:, c:c + 1],
            in1=xc[c], op0=ADD, op1=ADD)

    # ---------------- Phase H: outputs -----------------------------------
    for c in range(CC):
        nc.gpsimd.dma_start(out=prm['x_out'][b, c * 128:(c + 1) * 128, :],
                            in_=xc[c][:, 1:N])
        nc.gpsimd.dma_start(out=prm['cls_out'][b, c * 128:(c + 1) * 128],
                            in_=xc[c][:, 0:1])


def _register_const(nc, dtype, value):
    t = nc.alloc_sbuf_tensor(f"const-{dtype.name}-{value}", [128, 1], dtype)
    nc.gpsimd.memset(t.ap(), value)
    nc.const_aps.aps[(dtype, value)] = t.ap()


def build_program():
    nc = bass.Bass()
    _register_const(nc, mybir.dt.float32, 1e-5)
    nc.all_engine_barrier()
    prm = {}
    dp = nc.declare_dram_parameter
    prm['xpad_bf'] = dp("xpad_bf", [BPC, C, PADW * PADW], BF, isOutput=False)
    prm['x_f32'] = dp("x_f32", [BPC, C, HWP], F32, isOutput=False)
    prm['cls'] = dp("cls", [BPC, C, 1], F32, isOutput=False)
    prm['dconv'] = dp("dconv", [CC, 128, 9, 128], BF, isOutput=False)
    prm['conv_b'] = dp("conv_b", [128, CC], F32, isOutput=False)
    prm['wq'] = dp("wq", [C, QROWS], BF, isOutput=False)
    prm['wk'] = dp("wk", [C, QROWS], BF, isOutput=False)
    prm['wv'] = dp("wv", [C, C], BF, isOutput=False)
    prm['bq'] = dp("bq", [128, QROWS // 128], F32, isOutput=False)
    prm['bk'] = dp("bk", [128, QROWS // 128], F32, isOutput=False)
    prm['bv_bcast'] = dp("bv_bcast", [128, C], F32, isOutput=False)
    prm['wproj'] = dp("wproj", [HEADS, D + 1, C], BF, isOutput=False)
    prm['bproj'] = dp("bproj", [128, CC], F32, isOutput=False)
    prm['wfc1'] = dp("wfc1", [C, HID], BF, isOutput=False)
    prm['bfc1'] = dp("bfc1", [128, HC], F32, isOutput=False)
    prm['wfc2'] = dp("wfc2", [HID, C], BF, isOutput=False)
    prm['bfc2'] = dp("bfc2", [128, CC], F32, isOutput=False)
    prm['cls_out'] = dp("cls_out", [BPC, C, 1], F32, isOutput=True)
    prm['x_out'] = dp("x_out", [BPC, C, HWP], F32, isOutput=True)
    prm['gattn'] = dp("gattn", [BPC, HWP], F32, isOutput=True)

    with tile.TileContext(nc) as tc:
        import contextlib
        with contextlib.ExitStack() as ctx:
            wpool = ctx.enter_context(tc.tile_pool(name="wpool", bufs=1))
            abuf = ctx.enter_context(tc.tile_pool(name="abuf", bufs=1))
            inbuf = ctx.enter_context(tc.tile_pool(name="inbuf", bufs=2))
            scratch = ctx.enter_context(tc.tile_pool(name="scratch", bufs=3))
            ppool = ctx.enter_context(tc.tile_pool(name="ppool", bufs=3))
            pspool = ctx.enter_context(
                tc.tile_pool(name="pspool", bufs=4, space="PSUM"))
            dpool = ctx.enter_context(
                tc.tile_pool(name="dpool", bufs=1, space="DRAM"))
            pools = (wpool, abuf, inbuf, scratch, ppool, pspool, dpool)

            wts = {}
            dconv = []
            for c in range(CC):
                t = wpool.tile([128, 9, 128], BF, name=f"dconv{c}",
                               tag=f"dconv{c}")
                nc.scalar.dma_start(out=t, in_=prm['dconv'][c])
                dconv.append(t)
            wts['dconv'] = dconv
            for key, rows, width in (('wq', C, QROWS), ('wk', C, QROWS),
                                     ('wv', C, C), ('wfc1', C, HID)):
                ts = []
                for c in range(rows // 128):
                    t = wpool.tile([128, width], BF, name=f"{key}{c}",
                                   tag=f"{key}{c}")
                    nc.scalar.dma_start(out=t,
                                      in_=prm[key][c * 128:(c + 1) * 128])
                    ts.append(t)
                wts[key] = ts
            ts = []
            for j in range(HC):
                t = wpool.tile([128, C], BF, name=f"wfc2{j}", tag=f"wfc2{j}")
                nc.scalar.dma_start(out=t, in_=prm['wfc2'][j * 128:(j + 1) * 128])
                ts.append(t)
            wts['wfc2'] = ts
            ts = []
            for hh in range(HEADS):
                t = wpool.tile([D + 1, C], BF, name=f"wproj{hh}",
                               tag=f"wproj{hh}")
                nc.scalar.dma_start(out=t, in_=prm['wproj'][hh])
                ts.append(t)
            wts['wproj'] = ts
            for key, shape in (('conv_b', [128, CC]), ('bq', [128, 4]),
                               ('bk', [128, 4]), ('bproj', [128, CC]),
                               ('bfc1', [128, HC]), ('bfc2', [128, CC]),
                               ('bv_bcast', [128, C])):
                t = wpool.tile(shape, F32, name=f"w_{key}", tag=f"w_{key}")
                nc.scalar.dma_start(out=t, in_=prm[key][:, :])
                wts[key] = t
            ones_col = wpool.tile([128, 1], BF, name="ones_col",
                                  tag="ones_col")
            nc.vector.memset(ones_col, 1.0)
            wts['ones_col'] = ones_col
            ones_row = wpool.tile([1, 128], BF, name="ones_row",
                                  tag="ones_row")
            nc.vector.memset(ones_row, 1.0)
            wts['ones_row'] = ones_row

            for b in range(BPC):
                _emit_batch(nc, tc, pools, wts, prm, b)
    _split_excess_waits(nc)
    return nc, prm


_CACHED = None


def _get_program():
    global _CACHED
    if _CACHED is None:
        _CACHED = build_program()
    return _CACHED


def _prep_host(inputs):
    """Fold LN affine into weights, transpose/pad into lhsT layouts, pad x
    for SAME conv, cast matmul operands to bf16."""
    f32 = np.float32
    x = np.asarray(inputs['x'], f32)
    cls_token = np.asarray(inputs['cls_token'], f32)
    conv_w = np.asarray(inputs['conv_w'], f32)
    conv_b = np.asarray(inputs['conv_b'], f32)
    ln1_g = np.asarray(inputs['ln1_g'], f32)
    ln1_b = np.asarray(inputs['ln1_b'], f32)
    qkv_w = np.asarray(inputs['qkv_w'], f32)
    proj_w = np.asarray(inputs['proj_w'], f32)
    proj_b = np.asarray(inputs['proj_b'], f32)
    ln2_g = np.asarray(inputs['ln2_g'], f32)
    ln2_b = np.asarray(inputs['ln2_b'], f32)
    fc1_w = np.asarray(inputs['fc1_w'], f32)
    fc1_b = np.asarray(inputs['fc1_b'], f32)
    fc2_w = np.asarray(inputs['fc2_w'], f32)
    fc2_b = np.asarray(inputs['fc2_b'], f32)

    def colmajor(v):          # [C*] vector -> [128, C*/128] chunk-col layout
        return np.ascontiguousarray(v.reshape(-1, 128).T).astype(f32)

    shared = {}
    wt = conv_w[:, 0].reshape(C, 9)
    dconv = np.zeros((CC, 128, 9, 128), f32)
    idx = np.arange(128)
    for c in range(CC):
        dconv[c, idx, :, idx] = wt[c * 128:(c + 1) * 128, :]
    shared['dconv'] = dconv.astype(BF16)
    shared['conv_b'] = colmajor(conv_b)

    qkv_g = qkv_w * ln1_g[None, :]
    qkv_bias = qkv_w @ ln1_b
    wq_f, wk_f, wv_f = qkv_g[0:C], qkv_g[C:2 * C], qkv_g[2 * C:3 * C]
    bq_f, bk_f, bv_f = qkv_bias[0:C], qkv_bias[C:2 * C], qkv_bias[2 * C:3 * C]

    def pad_qk(wf):
        out = np.zeros((C, QROWS), f32)
        for hh in range(HEADS):
            out[:, hh * QP: hh * QP + D] = wf[hh * D:(hh + 1) * D, :].T
        return out

    def pad_qk_bias(bf_):
        out = np.zeros((QROWS,), f32)
        for hh in range(HEADS):
            out[hh * QP: hh * QP + D] = bf_[hh * D:(hh + 1) * D]
        return colmajor(out)

    shared['wq'] = pad_qk(wq_f).astype(BF16)
    shared['wk'] = pad_qk(wk_f).astype(BF16)
    shared['bq'] = pad_qk_bias(bq_f)
    shared['bk'] = pad_qk_bias(bk_f)
    shared['wv'] = wv_f.T.copy().astype(BF16)
    shared['bv_bcast'] = np.tile(bv_f[None, :], (128, 1)).astype(f32)
    wproj = np.zeros((HEADS, D + 1, C), f32)
    wproj[:, 1:, :] = proj_w.T.reshape(HEADS, D, C)
    shared['wproj'] = wproj.astype(BF16)
    shared['bproj'] = colmajor(proj_b)
    shared['wfc1'] = (fc1_w * ln2_g[None, :]).T.copy().astype(BF16)
    shared['bfc1'] = colmajor(fc1_b + fc1_w @ ln2_b)
    shared['wfc2'] = fc2_w.T.copy().astype(BF16)
    shared['bfc2'] = colmajor(fc2_b)

    xf = x.reshape(B, C, HWP)
    xpad = np.zeros((B, C, PADW, PADW), f32)
    xpad[:, :, 1:1 + H, 1:1 + W] = x
    xpad_bf = xpad.reshape(B, C, PADW * PADW).astype(BF16)
    clsr = np.ascontiguousarray(cls_token[:, 0, :]).reshape(B, C, 1)

    in_maps = []
    for core in range(NCORES):
        sl = slice(core * BPC, (core + 1) * BPC)
        m = dict(shared)
        m['xpad_bf'] = np.ascontiguousarray(xpad_bf[sl])
        m['x_f32'] = np.ascontiguousarray(xf[sl])
        m['cls'] = np.ascontiguousarray(clsr[sl])
        in_maps.append(m)
    return in_maps


def kernel(**inputs):
    global LAST_EXEC_NS
    from concourse.bass_utils import run_bass_kernel_spmd
    nc, prm = _get_program()
    in_maps = _prep_host(inputs)
    res = run_bass_kernel_spmd(nc, in_maps, core_ids=list(range(NCORES)),
                               trace=TRACE)
    LAST_EXEC_NS = res.exec_time_ns
    cls_out = np.zeros((B, 1, C), np.float32)
    x_out = np.zeros((B, C, H, W), np.float32)
    gattn = np.zeros((B, HWP), np.float32)
    for core in range(NCORES):
        r = res.results[core]
        sl = slice(core * BPC, (core + 1) * BPC)
        cls_out[sl, 0, :] = r['cls_out'][:, :, 0]
        x_out[sl] = r['x_out'].reshape(BPC, C, H, W)
        gattn[sl] = r['gattn']
    return cls_out, x_out, gattn


# revision 33
# speedup vs baseline: 1.1371x; 1.0069x over previous
"""Trainium2 Bass kernel for the EvoSA block (depthwise-conv positional
encoding + attention with global_attn stats + MLP).

Self-contained: takes FULL inputs as in reference.setup_inputs(), shards
batch B=16 across 8 NeuronCores (2 per core), returns FULL outputs
(cls_out, x_out, global_attn).

Layout strategy (per core, 2 batch elements):
  - activations channel-major [C, N] (channels on partitions, tokens free)
  - LN affine folded into the following matmul weights on host
  - depthwise conv as 9 diagonal-weight matmuls accumulating in PSUM
  - attention scores computed transposed S^T[m,n] = k_m.q_n so softmax's
    denominator comes from a ones-column appended to V in the attn@v matmul
  - exp without max-subtraction (scores are ~N(0,0.15); overflow impossible)
"""
import sys

sys.path.insert(0, '/opt/trn_rl_repo')

import numpy as np
import ml_dtypes

import concourse.bass as bass
import concourse.tile as tile
from concourse import mybir
from concourse.vector_clock import ScopedClock, VectorClock

BF16 = ml_dtypes.bfloat16

# ---------------------------------------------------------------------------
# Environment patches.
# (1) walrus on this build accepts only one sync-wait per CTRL instruction:
#     split the TileContext tail-drain into single-wait drains.
# (2) the trimmed repo lacks antenv.axon_hooks; recreate it so
#     run_bass_kernel_spmd(trace=True) can profile via NTFF.
# ---------------------------------------------------------------------------


def _drain_and_barrier_split(self, tick_clock, wait_clock):
    gc = list(tick_clock.global_clock)
    nonzero = [i for i, t in enumerate(gc) if t > 0]
    for i in nonzero:
        sub = [gc[j] if j == i else 0 for j in range(len(gc))]
        drain_inst = self.nc.sync.drain()
        wait_clock.add_sem_waits(drain_inst.ins,
                                 ScopedClock({None: VectorClock(sub)}))
    if not nonzero:
        self.nc.sync.drain()
    self.nc.all_engine_barrier()
    assert self.sems is not None
    popped = self.nc._tile_sem_poison_stack.pop()
    assert popped is self._sem_poison
    self.nc.clear_and_free_semaphores(list(self.sems.allocated().values()))
    self.nc.all_engine_barrier()


tile.TileContext._drain_and_barrier = _drain_and_barrier_split


def _split_excess_waits(nc, max_waits=1):
    """This walrus build accepts only one sync-wait command per instruction.
    Move excess waits onto preceding same-engine NOPs."""
    import bass_rust
    nsplit = 0
    for bb in nc.main_func.blocks:
        out = []
        changed = False
        for ins in bb.instructions:
            si = ins.sync_info
            waits = list(si.on_wait) if si is not None and si.on_wait else []
            # DMA waits are queue-level (descriptor) waits; moving them to a
            # sequencer NOP would stall the whole queue-push stream and can
            # deadlock. Leave them alone.

            if len(waits) > max_waits:
                extra, keep = waits[:-max_waits], waits[-max_waits:]
                for k, i0 in enumerate(range(0, len(extra), max_waits)):
                    nop = mybir.InstNoOp(name=f"{ins.name}-ws{k}", ins=[],
                                         outs=[])
                    nop.engine = ins.engine
                    nop.sync_info = bass_rust.SyncInfo(
                        on_wait=extra[i0:i0 + max_waits], on_update=[])
                    out.append(nop)
                    nsplit += 1
                ins.sync_info = bass_rust.SyncInfo(
                    on_wait=keep, on_update=list(si.on_update))
                changed = True
            out.append(ins)
        if changed:
            bb.instructions = out
    return nsplit


def _install_ntff_hook():
    import types
    try:
        import antenv
        if hasattr(antenv, 'axon_hooks'):
            return
        mod = types.ModuleType('antenv.axon_hooks')
        _h = [None]
        mod.set_axon_ntff_profile_hook = lambda h: _h.__setitem__(0, h)
        mod.get_axon_ntff_profile_hook = lambda: _h[0]
        sys.modules['antenv.axon_hooks'] = mod
        antenv.axon_hooks = mod
        from trn_agent_boot.trn_boot import _ntff_profile_via_ctypes
        mod.set_axon_ntff_profile_hook(
            _ntff_profile_via_ctypes('/opt/axon/libaxon_pjrt.so'))
    except Exception:
        pass


_install_ntff_hook()

# ---------------------------------------------------------------------------
# Problem constants (hardcoded per contract)
# ---------------------------------------------------------------------------
B, C, H, W = 16, 384, 28, 28
HEADS = 8
D = C // HEADS                   # 48
N = H * W + 1                    # 785 tokens (cls + 784 patches)
HWP = H * W                      # 784
NCORES = 8
BPC = B // NCORES                # 2 batches per core
CC = C // 128                    # 3 channel chunks
QP = 64                          # per-head padded width in q/k layout
QROWS = HEADS * QP               # 512
HID = 4 * C                      # 1536
HC = HID // 128                  # 12
SCALE = float(D) ** -0.5
PADW = 30                        # padded spatial width (30x30)

NSPLIT = [(0, 512), (512, N - 512)]          # matmul N<=512 splits of 785
TOKC = [(i * 128, min(128, N - i * 128)) for i in range((N + 127) // 128)]

F32 = mybir.dt.float32
BF = mybir.dt.bfloat16
ADD = mybir.AluOpType.add
MULT = mybir.AluOpType.mult

TRACE = False          # set by test harness for profiled runs
LAST_EXEC_NS = None


def _pbcast(nc, dpool, src, dst, which):
    """Broadcast src [1, F] to dst [P, F] across partitions by bouncing
    through DRAM (DRAM source APs allow a zero partition step; SBUF ones
    don't, and this walrus build can't codegen gpsimd partition_broadcast)."""
    f = src.shape[-1]
    dtmp = dpool.tile([1, f], src.dtype, name=f"dtmp{which}", tag="dtmp",
                      bufs=4)
    nc.gpsimd.dma_start(out=dtmp, in_=src)
    nc.gpsimd.dma_start(out=dst, in_=bass.AP(
        tensor=dtmp.tensor, offset=dtmp.offset,
        ap=[[0, dst.shape[0]]] + [list(dtmp.ap[-1])]))


def _layernorm(nc, pools, wts, xc, which):
    """y = (xc - mu) * rstd in bf16, channel-major. LN affine is folded into
    the consuming matmul weights on the host."""
    wpool, abuf, inbuf, scratch, ppool, pspool, dpool = pools
    s1 = pspool.tile([1, N], F32, name=f"s1{which}", tag="ps")
    s2 = pspool.tile([1, N], F32, name=f"s2{which}", tag="ps")
    for c in range(CC):
        xb = scratch.tile([128, N], BF, name=f"xb{which}{c}", tag="scratch_bf")
        nc.vector.tensor_copy(out=xb, in_=xc[c])
        sq = scratch.tile([128, N], BF, name=f"sq{which}{c}", tag="scratch_bf")
        nc.vector.tensor_mul(out=sq, in0=xc[c], in1=xc[c])
        for (o, w) in NSPLIT:
            nc.tensor.matmul(s1[:, o:o + w], wts['ones_col'], xb[:, o:o + w],
                             start=(c == 0), stop=(c == CC - 1))
            nc.tensor.matmul(s2[:, o:o + w], wts['ones_col'], sq[:, o:o + w],
                             start=(c == 0), stop=(c == CC - 1))
    # var*C = s2 - s1^2/C ; std = sqrt(var*C/C + eps); rstd = 1/std
    t1 = scratch.tile([1, N], F32, name=f"t1{which}", tag="ln_small")
    nc.scalar.square(out=t1, in_=s1)
    v384 = scratch.tile([1, N], F32, name=f"v384{which}", tag="ln_small")
    nc.vector.scalar_tensor_tensor(out=v384, in0=t1, scalar=-1.0 / C, in1=s2,
                                   op0=MULT, op1=ADD)
    std = scratch.tile([1, N], F32, name=f"std{which}", tag="ln_small")
    nc.scalar.activation(out=std, in_=v384,
                         func=mybir.ActivationFunctionType.Sqrt,
                         bias=1e-5, scale=1.0 / C)
    rstd = scratch.tile([1, N], BF, name=f"rstd{which}", tag="ln_small")
    with nc.allow_low_precision(reason="rstd feeds bf16 y anyway"):
        nc.vector.reciprocal(out=rstd, in_=std)
    nm = scratch.tile([1, N], BF, name=f"nm{which}", tag="ln_small")
    nc.vector.scalar_tensor_tensor(out=nm, in0=s1, scalar=-1.0 / C, in1=rstd,
                                   op0=MULT, op1=MULT)
    # broadcast rstd and nm across partitions via K=1 ones-outer matmuls
    # (keeps the LN critical path on PE; avoids DMA-ring round trips)
    rbp = pspool.tile([128, N], F32, name=f"rbp{which}", tag="ps")
    nbp = pspool.tile([128, N], F32, name=f"nbp{which}", tag="ps")
    for (o, w) in NSPLIT:
        nc.tensor.matmul(rbp[:, o:o + w], wts['ones_row'], rstd[:, o:o + w],
                         start=True, stop=True)
        nc.tensor.matmul(nbp[:, o:o + w], wts['ones_row'], nm[:, o:o + w],
                         start=True, stop=True)
    y = []
    for c in range(CC):
        yt = scratch.tile([128, N], BF, name=f"yt{which}{c}", tag="scratch_bf")
        nc.vector.tensor_mul(out=yt, in0=xc[c], in1=rbp)
        yb = abuf.tile([128, N], BF, name=f"y{which}{c}", tag=f"y{c}", bufs=2)
        nc.vector.tensor_add(out=yb, in0=yt, in1=nbp)
        y.append(yb)
    return y


def _emit_batch(nc, tc, pools, wts, prm, b):
    wpool, abuf, inbuf, scratch, ppool, pspool, dpool = pools

    # ---------------- Phase A: conv + residual + xc^T assembly ------------
    xpad, xf32 = [], []
    for c in range(CC):
        xpad_t = inbuf.tile([128, PADW, PADW], BF, name=f"xpad{c}",
                            tag=f"xpad{c}")
        nc.sync.dma_start(out=xpad_t,
                          in_=prm['xpad_bf'][b, c * 128:(c + 1) * 128]
                          .rearrange("p (h w) -> p h w", h=PADW))
        xpad.append(xpad_t)
        xf_t = inbuf.tile([128, HWP], F32, name=f"xf{c}", tag=f"xf{c}")
        nc.sync.dma_start(out=xf_t, in_=prm['x_f32'][b, c * 128:(c + 1) * 128])
        xf32.append(xf_t)

    xc = []
    for c in range(CC):
        xc_t = abuf.tile([128, N], F32, name=f"xc{c}", tag=f"xc{c}", bufs=2)
        nc.sync.dma_start(out=xc_t[:, 0:1],
                          in_=prm['cls'][b, c * 128:(c + 1) * 128])
        for half in range(2):
            cps = pspool.tile([128, 392], F32, name=f"cps{c}{half}", tag="ps")
            r0 = half * 14
            for t in range(9):
                ti, tj = divmod(t, 3)
                rhs = xpad[c][:, ti + r0: ti + r0 + 14, tj: tj + W]
                nc.tensor.matmul(cps, wts['dconv'][c][:, t, :], rhs,
                                 start=(t == 0), stop=(t == 8))
            nc.vector.scalar_tensor_tensor(
                out=xc_t[:, 1 + half * 392: 1 + (half + 1) * 392], in0=cps,
                scalar=wts['conv_b'][:, c:c + 1],
                in1=xf32[c][:, half * 392:(half + 1) * 392], op0=ADD, op1=ADD)
        xc.append(xc_t)

    # ---------------- Phase B: LN1 -> y ----------------------------------
    y = _layernorm(nc, pools, wts, xc, f"a{b}")

    # ---------------- Phase C: q^T, k^T (head-padded), v (token-major) ---
    qT, kT = [], []
    for name, wkey, bkey, dst in (("q", 'wq', 'bq', qT), ("k", 'wk', 'bk', kT)):
        for j in range(QROWS // 128):
            ps = pspool.tile([128, N], F32, name=f"ps{name}{j}", tag="ps")
            for c in range(CC):
                for (o, w) in NSPLIT:
                    nc.tensor.matmul(ps[:, o:o + w],
                                     wts[wkey][c][:, j * 128:(j + 1) * 128],
                                     y[c][:, o:o + w],
                                     start=(c == 0), stop=(c == CC - 1))
            ot = abuf.tile([128, N], BF, name=f"{name}T{j}", tag=f"{name}T{j}")
            nc.vector.tensor_scalar_add(out=ot, in0=ps,
                                        scalar1=wts[bkey][:, j:j + 1])
            dst.append(ot)

    v_sb = []
    for mc, (ms, mr) in enumerate(TOKC):
        ps = pspool.tile([128, C], F32, name=f"psv{mc}", tag="ps")
        for c in range(CC):
            nc.tensor.matmul(ps[:mr, :], y[c][:, ms:ms + mr], wts['wv'][c],
                             start=(c == 0), stop=(c == CC - 1))
        vt = abuf.tile([128, HEADS, D + 1], BF, name=f"v{mc}", tag=f"v{mc}")
        nc.gpsimd.memset(vt[:mr, :, 0:1], 1.0)
        nc.vector.scalar_tensor_tensor(
            out=vt[:mr, :, 1:D + 1],
            in0=ps[:mr, :].rearrange("p (h d) -> p h d", h=HEADS),
            scalar=1.0,
            in1=wts['bv_bcast'][:mr, :].rearrange("p (h d) -> p h d", h=HEADS),
            op0=MULT, op1=ADD)
        v_sb.append(vt)

    # ---------------- Phase D: attention, head at a time ------------------
    o_bf = []
    r0_row = abuf.tile([1, HEADS], BF, name="r0row", tag="r0row")
    gcols = [abuf.tile([128, HEADS], BF, name=f"gcol{mc}", tag=f"gcol{mc}")
             for mc in range(len(TOKC))]

    for h in range(HEADS):
        jt, jr = h // 2, QP * (h % 2)
        ops = pspool.tile([D + 1, N], F32, name=f"ops{h}", tag="ps")
        for mc, (ms, mr) in enumerate(TOKC):
            sps = pspool.tile([128, N], F32, name=f"sps{h}{mc}", tag="ps")
            for (o, w) in NSPLIT:
                nc.tensor.matmul(sps[:mr, o:o + w],
                                 kT[jt][jr:jr + QP, ms:ms + mr],
                                 qT[jt][jr:jr + QP, o:o + w],
                                 start=True, stop=True)
            pt = ppool.tile([128, N], BF, name=f"p{h}{mc}", tag="pt")
            nc.scalar.activation(out=pt[:mr, :], in_=sps[:mr, :],
                                 func=mybir.ActivationFunctionType.Exp,
                                 scale=SCALE)
            nc.gpsimd.tensor_copy(out=gcols[mc][:mr, h:h + 1],
                                  in_=pt[:mr, 0:1])
            for (o, w) in NSPLIT:
                nc.tensor.matmul(ops[:, o:o + w], v_sb[mc][:mr, h, :],
                                 pt[:mr, o:o + w],
                                 start=(mc == 0), stop=(mc == len(TOKC) - 1))
        rec = abuf.tile([1, N], BF, name=f"rec{h}", tag="rec", bufs=2)
        with nc.allow_low_precision(reason="softmax recip consumed in bf16"):
            nc.vector.reciprocal(out=rec, in_=ops[0:1, :])
        nc.gpsimd.tensor_copy(out=r0_row[:, h:h + 1], in_=rec[:, 0:1])
        rbs = scratch.tile([D + 1, N], BF, name=f"rbs{h}", tag="rbs", bufs=2)
        _pbcast(nc, pools[-1], rec, rbs, f"rec{h}")
        ob = abuf.tile([D + 1, N], BF, name=f"ob{h}", tag=f"ob{h}")
        nc.vector.tensor_mul(out=ob, in0=ops, in1=rbs)
        o_bf.append(ob)

    # ---------------- global_attn ----------------------------------------
    r0b = pspool.tile([128, HEADS], F32, name="r0b", tag="ps")
    nc.tensor.matmul(r0b, wts['ones_row'], r0_row, start=True, stop=True)
    for mc, (ms, mr) in enumerate(TOKC):
        gs = abuf.tile([128, 1], F32, name=f"gs{mc}", tag=f"gs{mc}")
        nc.vector.scalar_tensor_tensor(
            out=gcols[mc][:mr, :], in0=gcols[mc][:mr, :], scalar=1.0 / HEADS,
            in1=r0b[:mr, :], op0=MULT, op1=MULT, accum_out=gs[:mr, :])
        if mc == 0:
            nc.gpsimd.dma_start(out=prm['gattn'][b, 0:mr - 1], in_=gs[1:mr, :])
        else:
            nc.gpsimd.dma_start(out=prm['gattn'][b, ms - 1:ms - 1 + mr],
                                in_=gs[:mr, :])

    # ---------------- Phase E: proj + residual ---------------------------
    for c in range(CC):
        ps = pspool.tile([128, N], F32, name=f"psp{c}", tag="ps")
        for hh in range(HEADS):
            for (o, w) in NSPLIT:
                nc.tensor.matmul(ps[:, o:o + w],
                                 wts['wproj'][hh][:, c * 128:(c + 1) * 128],
                                 o_bf[hh][:, o:o + w],
                                 start=(hh == 0), stop=(hh == HEADS - 1))
        nc.vector.scalar_tensor_tensor(
            out=xc[c], in0=ps, scalar=wts['bproj'][:, c:c + 1],
            in1=xc[c], op0=ADD, op1=ADD)

    # ---------------- Phase F/G: LN2 + MLP -------------------------------
    y2 = _layernorm(nc, pools, wts, xc, f"b{b}")
    h_bf = []
    for j in range(HC):
        ps = pspool.tile([128, N], F32, name=f"psh{j}", tag="ps")
        for c in range(CC):
            for (o, w) in NSPLIT:
                nc.tensor.matmul(ps[:, o:o + w],
                                 wts['wfc1'][c][:, j * 128:(j + 1) * 128],
                                 y2[c][:, o:o + w],
                                 start=(c == 0), stop=(c == CC - 1))
        ht = abuf.tile([128, N], BF, name=f"h{j}", tag=f"h{j}")
        nc.scalar.activation(out=ht, in_=ps,
                             func=mybir.ActivationFunctionType.Gelu,
                             bias=wts['bfc1'][:, j:j + 1], scale=1.0)
        h_bf.append(ht)
    for c in range(CC):
        ps = pspool.tile([128, N], F32, name=f"psf{c}", tag="ps")
        for j in range(HC):
            for (o, w) in NSPLIT:
                nc.tensor.matmul(ps[:, o:o + w],
                                 wts['wfc2'][j][:, c * 128:(c + 1) * 128],
                                 h_bf[j][:, o:o + w],
                                 start=(j == 0), stop=(j == HC - 1))
        nc.vector.scalar_tensor_tensor(
            out=xc[c], in0=ps, scalar=wts['bfc2'][:, c:c + 1],
            in1=xc[c], op0=ADD, op1=ADD)

    # ---------------- Phase H: outputs -----------------------------------
    for c in range(CC):
        nc.gpsimd.dma_start(out=prm['x_out'][b, c * 128:(c + 1) * 128, :],
                            in_=xc[c][:, 1:N])
        nc.gpsimd.dma_start(out=prm['cls_out'][b, c * 128:(c + 1) * 128],
                            in_=xc[c][:, 0:1])


def _register_const(nc, dtype, value):
    t = nc.alloc_sbuf_tensor(f"const-{dtype.name}-{value}", [128, 1], dtype)
    nc.gpsimd.memset(t.ap(), value)
    nc.const_aps.aps[(dtype, value)] = t.ap()


def build_program():
    nc = bass.Bass()
    _register_const(nc, mybir.dt.float32, 1e-5)
    nc.all_engine_barrier()
    prm = {}
    dp = nc.declare_dram_parameter
    prm['xpad_bf'] = dp("xpad_bf", [BPC, C, PADW * PADW], BF, isOutput=False)
    prm['x_f32'] = dp("x_f32", [BPC, C, HWP], F32, isOutput=False)
    prm['cls'] = dp("cls", [BPC, C, 1], F32, isOutput=False)
    prm['dconv'] = dp("dconv", [CC, 128, 9, 128], BF, isOutput=False)
    prm['conv_b'] = dp("conv_b", [128, CC], F32, isOutput=False)
    prm['wq'] = dp("wq", [C, QROWS], BF, isOutput=False)
    prm['wk'] = dp("wk", [C, QROWS], BF, isOutput=False)
    prm['wv'] = dp("wv", [C, C], BF, isOutput=False)
    prm['bq'] = dp("bq", [128, QROWS // 128], F32, isOutput=False)
    prm['bk'] = dp("bk", [128, QROWS // 128], F32, isOutput=False)
    prm['bv_bcast'] = dp("bv_bcast", [128, C], F32, isOutput=False)
    prm['wproj'] = dp("wproj", [HEADS, D + 1, C], BF, isOutput=False)
    prm['bproj'] = dp("bproj", [128, CC], F32, isOutput=False)
    prm['wfc1'] = dp("wfc1", [C, HID], BF, isOutput=False)
    prm['bfc1'] = dp("bfc1", [128, HC], F32, isOutput=False)
    prm['wfc2'] = dp("wfc2", [HID, C], BF, isOutput=False)
    prm['bfc2'] = dp("bfc2", [128, CC], F32, isOutput=False)
    prm['cls_out'] = dp("cls_out", [BPC, C, 1], F32, isOutput=True)
    prm['x_out'] = dp("x_out", [BPC, C, HWP], F32, isOutput=True)
    prm['gattn'] = dp("gattn", [BPC, HWP], F32, isOutput=True)

    with tile.TileContext(nc) as tc:
        import contextlib
        with contextlib.ExitStack() as ctx:
            wpool = ctx.enter_context(tc.tile_pool(name="wpool", bufs=1))
            abuf = ctx.enter_context(tc.tile_pool(name="abuf", bufs=1))
            inbuf = ctx.enter_context(tc.tile_pool(name="inbuf", bufs=2))
            scratch = ctx.enter_context(tc.tile_pool(name="scratch", bufs=3))
            ppool = ctx.enter_context(tc.tile_pool(name="ppool", bufs=3))
            pspool = ctx.enter_context(
                tc.tile_pool(name="pspool", bufs=4, space="PSUM"))
            dpool = ctx.enter_context(
                tc.tile_pool(name="dpool", bufs=1, space="DRAM"))
            pools = (wpool, abuf, inbuf, scratch, ppool, pspool, dpool)

            wts = {}
            dconv = []
            for c in range(CC):
                t = wpool.tile([128, 9, 128], BF, name=f"dconv{c}",
                               tag=f"dconv{c}")
                nc.sync.dma_start(out=t, in_=prm['dconv'][c])
                dconv.append(t)
            wts['dconv'] = dconv
            for key, rows, width in (('wq', C, QROWS), ('wk', C, QROWS),
                                     ('wv', C, C), ('wfc1', C, HID)):
                ts = []
                for c in range(rows // 128):
                    t = wpool.tile([128, width], BF, name=f"{key}{c}",
                                   tag=f"{key}{c}")
                    nc.sync.dma_start(out=t,
                                      in_=prm[key][c * 128:(c + 1) * 128])
                    ts.append(t)
                wts[key] = ts
            ts = []
            for j in range(HC):
                t = wpool.tile([128, C], BF, name=f"wfc2{j}", tag=f"wfc2{j}")
                nc.sync.dma_start(out=t, in_=prm['wfc2'][j * 128:(j + 1) * 128])
                ts.append(t)
            wts['wfc2'] = ts
            ts = []
            for hh in range(HEADS):
                t = wpool.tile([D + 1, C], BF, name=f"wproj{hh}",
                               tag=f"wproj{hh}")
                nc.sync.dma_start(out=t, in_=prm['wproj'][hh])
                ts.append(t)
            wts['wproj'] = ts
            for key, shape in (('conv_b', [128, CC]), ('bq', [128, 4]),
                               ('bk', [128, 4]), ('bproj', [128, CC]),
                               ('bfc1', [128, HC]), ('bfc2', [128, CC]),
                               ('bv_bcast', [128, C])):
                t = wpool.tile(shape, F32, name=f"w_{key}", tag=f"w_{key}")
                nc.sync.dma_start(out=t, in_=prm[key][:, :])
                wts[key] = t
            ones_col = wpool.tile([128, 1], BF, name="ones_col",
                                  tag="ones_col")
            nc.vector.memset(ones_col, 1.0)
            wts['ones_col'] = ones_col
            ones_row = wpool.tile([1, 128], BF, name="ones_row",
                                  tag="ones_row")
            nc.vector.memset(ones_row, 1.0)
            wts['ones_row'] = ones_row

            for b in range(BPC):
                _emit_batch(nc, tc, pools, wts, prm, b)
    _split_excess_waits(nc)
    return nc, prm


_CACHED = None


def _get_program():
    global _CACHED
    if _CACHED is None:
        _CACHED = build_program()
    return _CACHED


def _prep_host(inputs):
    """Fold LN affine into weights, transpose/pad into lhsT layouts, pad x
    for SAME conv, cast matmul operands to bf16."""
    f32 = np.float32
    x = np.asarray(inputs['x'], f32)
    cls_token = np.asarray(inputs['cls_token'], f32)
    conv_w = np.asarray(inputs['conv_w'], f32)
    conv_b = np.asarray(inputs['conv_b'], f32)
    ln1_g = np.asarray(inputs['ln1_g'], f32)
    ln1_b = np.asarray(inputs['ln1_b'], f32)
    qkv_w = np.asarray(inputs['qkv_w'], f32)
    proj_w = np.asarray(inputs['proj_w'], f32)
    proj_b = np.asarray(inputs['proj_b'], f32)
    ln2_g = np.asarray(inputs['ln2_g'], f32)
    ln2_b = np.asarray(inputs['ln2_b'], f32)
    fc1_w = np.asarray(inputs['fc1_w'], f32)
    fc1_b = np.asarray(inputs['fc1_b'], f32)
    fc2_w = np.asarray(inputs['fc2_w'], f32)
    fc2_b = np.asarray(inputs['fc2_b'], f32)

    def colmajor(v):          # [C*] vector -> [128, C*/128] chunk-col layout
        return np.ascontiguousarray(v.reshape(-1, 128).T).astype(f32)

    shared = {}
    wt = conv_w[:, 0].reshape(C, 9)
    dconv = np.zeros((CC, 128, 9, 128), f32)
    idx = np.arange(128)
    for c in range(CC):
        dconv[c, idx, :, idx] = wt[c * 128:(c + 1) * 128, :]
    shared['dconv'] = dconv.astype(BF16)
    shared['conv_b'] = colmajor(conv_b)

    qkv_g = qkv_w * ln1_g[None, :]
    qkv_bias = qkv_w @ ln1_b
    wq_f, wk_f, wv_f = qkv_g[0:C], qkv_g[C:2 * C], qkv_g[2 * C:3 * C]
    bq_f, bk_f, bv_f = qkv_bias[0:C], qkv_bias[C:2 * C], qkv_bias[2 * C:3 * C]

    def pad_qk(wf):
        out = np.zeros((C, QROWS), f32)
        for hh in range(HEADS):
            out[:, hh * QP: hh * QP + D] = wf[hh * D:(hh + 1) * D, :].T
        return out

    def pad_qk_bias(bf_):
        out = np.zeros((QROWS,), f32)
        for hh in range(HEADS):
            out[hh * QP: hh * QP + D] = bf_[hh * D:(hh + 1) * D]
        return colmajor(out)

    shared['wq'] = pad_qk(wq_f).astype(BF16)
    shared['wk'] = pad_qk(wk_f).astype(BF16)
    shared['bq'] = pad_qk_bias(bq_f)
    shared['bk'] = pad_qk_bias(bk_f)
    shared['wv'] = wv_f.T.copy().astype(BF16)
    shared['bv_bcast'] = np.tile(bv_f[None, :], (128, 1)).astype(f32)
    wproj = np.zeros((HEADS, D + 1, C), f32)
    wproj[:, 1:, :] = proj_w.T.reshape(HEADS, D, C)
    shared['wproj'] = wproj.astype(BF16)
    shared['bproj'] = colmajor(proj_b)
    shared['wfc1'] = (fc1_w * ln2_g[None, :]).T.copy().astype(BF16)
    shared['bfc1'] = colmajor(fc1_b + fc1_w @ ln2_b)
    shared['wfc2'] = fc2_w.T.copy().astype(BF16)
    shared['bfc2'] = colmajor(fc2_b)

    xf = x.reshape(B, C, HWP)
    xpad = np.zeros((B, C, PADW, PADW), f32)
    xpad[:, :, 1:1 + H, 1:1 + W] = x
    xpad_bf = xpad.reshape(B, C, PADW * PADW).astype(BF16)
    clsr = np.ascontiguousarray(cls_token[:, 0, :]).reshape(B, C, 1)

    in_maps = []
    for core in range(NCORES):
        sl = slice(core * BPC, (core + 1) * BPC)
        m = dict(shared)
        m['xpad_bf'] = np.ascontiguousarray(xpad_bf[sl])
        m['x_f32'] = np.ascontiguousarray(xf[sl])
        m['cls'] = np.ascontiguousarray(clsr[sl])
        in_maps.append(m)
    return in_maps


def kernel(**inputs):
    global LAST_EXEC_NS
    from concourse.bass_utils import run_bass_kernel_spmd
    nc, prm = _get_program()
    in_maps = _prep_host(inputs)
    res = run_bass_kernel_spmd(nc, in_maps, core_ids=list(range(NCORES)),
                               trace=TRACE)
    LAST_EXEC_NS = res.exec_time_ns
    cls_out = np.zeros((B, 1, C), np.float32)
    x_out = np.zeros((B, C, H, W), np.float32)
    gattn = np.zeros((B, HWP), np.float32)
    for core in range(NCORES):
        r = res.results[core]
        sl = slice(core * BPC, (core + 1) * BPC)
        cls_out[sl, 0, :] = r['cls_out'][:, :, 0]
        x_out[sl] = r['x_out'].reshape(BPC, C, H, W)
        gattn[sl] = r['gattn']
    return cls_out, x_out, gattn


# revision 34
# speedup vs baseline: 1.2347x; 1.0858x over previous
"""Trainium2 Bass kernel for the EvoSA block (depthwise-conv positional
encoding + attention with global_attn stats + MLP).

Self-contained: takes FULL inputs as in reference.setup_inputs(), shards
batch B=16 across 8 NeuronCores (2 per core), returns FULL outputs
(cls_out, x_out, global_attn).

Layout strategy (per core, 2 batch elements):
  - activations channel-major [C, N] (channels on partitions, tokens free)
  - LN affine folded into the following matmul weights on host
  - depthwise conv as 9 diagonal-weight matmuls accumulating in PSUM
  - attention scores computed transposed S^T[m,n] = k_m.q_n so softmax's
    denominator comes from a ones-column appended to V in the attn@v matmul
  - exp without max-subtraction (scores are ~N(0,0.15); overflow impossible)
"""
import sys

sys.path.insert(0, '/opt/trn_rl_repo')

import numpy as np
import ml_dtypes

import concourse.bass as bass
import concourse.tile as tile
from concourse import mybir
from concourse.vector_clock import ScopedClock, VectorClock

BF16 = ml_dtypes.bfloat16

# ---------------------------------------------------------------------------
# Environment patches.
# (1) walrus on this build accepts only one sync-wait per CTRL instruction:
#     split the TileContext tail-drain into single-wait drains.
# (2) the trimmed repo lacks antenv.axon_hooks; recreate it so
#     run_bass_kernel_spmd(trace=True) can profile via NTFF.
# ---------------------------------------------------------------------------


def _drain_and_barrier_split(self, tick_clock, wait_clock):
    gc = list(tick_clock.global_clock)
    nonzero = [i for i, t in enumerate(gc) if t > 0]
    for i in nonzero:
        sub = [gc[j] if j == i else 0 for j in range(len(gc))]
        drain_inst = self.nc.sync.drain()
        wait_clock.add_sem_waits(drain_inst.ins,
                                 ScopedClock({None: VectorClock(sub)}))
    if not nonzero:
        self.nc.sync.drain()
    self.nc.all_engine_barrier()
    assert self.sems is not None
    popped = self.nc._tile_sem_poison_stack.pop()
    assert popped is self._sem_poison
    self.nc.clear_and_free_semaphores(list(self.sems.allocated().values()))
    self.nc.all_engine_barrier()


tile.TileContext._drain_and_barrier = _drain_and_barrier_split


def _split_excess_waits(nc, max_waits=1):
    """This walrus build accepts only one sync-wait command per instruction.
    Move excess waits onto preceding same-engine NOPs."""
    import bass_rust
    nsplit = 0
    for bb in nc.main_func.blocks:
        out = []
        changed = False
        for ins in bb.instructions:
            si = ins.sync_info
            waits = list(si.on_wait) if si is not None and si.on_wait else []
            # DMA waits are queue-level (descriptor) waits; moving them to a
            # sequencer NOP would stall the whole queue-push stream and can
            # deadlock. Leave them alone.

            if len(waits) > max_waits:
                extra, keep = waits[:-max_waits], waits[-max_waits:]
                for k, i0 in enumerate(range(0, len(extra), max_waits)):
                    nop = mybir.InstNoOp(name=f"{ins.name}-ws{k}", ins=[],
                                         outs=[])
                    nop.engine = ins.engine
                    nop.sync_info = bass_rust.SyncInfo(
                        on_wait=extra[i0:i0 + max_waits], on_update=[])
                    out.append(nop)
                    nsplit += 1
                ins.sync_info = bass_rust.SyncInfo(
                    on_wait=keep, on_update=list(si.on_update))
                changed = True
            out.append(ins)
        if changed:
            bb.instructions = out
    return nsplit


def _install_ntff_hook():
    import types
    try:
        import antenv
        if hasattr(antenv, 'axon_hooks'):
            return
        mod = types.ModuleType('antenv.axon_hooks')
        _h = [None]
        mod.set_axon_ntff_profile_hook = lambda h: _h.__setitem__(0, h)
        mod.get_axon_ntff_profile_hook = lambda: _h[0]
        sys.modules['antenv.axon_hooks'] = mod
        antenv.axon_hooks = mod
        from trn_agent_boot.trn_boot import _ntff_profile_via_ctypes
        mod.set_axon_ntff_profile_hook(
            _ntff_profile_via_ctypes('/opt/axon/libaxon_pjrt.so'))
    except Exception:
        pass


_install_ntff_hook()

# ---------------------------------------------------------------------------
# Problem constants (hardcoded per contract)
# ---------------------------------------------------------------------------
B, C, H, W = 16, 384, 28, 28
HEADS = 8
D = C // HEADS                   # 48
N = H * W + 1                    # 785 tokens (cls + 784 patches)
HWP = H * W                      # 784
NCORES = 8
BPC = B // NCORES                # 2 batches per core
CC = C // 128                    # 3 channel chunks
QP = 64                          # per-head padded width in q/k layout
QROWS = HEADS * QP               # 512
HID = 4 * C                      # 1536
HC = HID // 128                  # 12
SCALE = float(D) ** -0.5
PADW = 30                        # padded spatial width (30x30)

NSPLIT = [(0, 512), (512, N - 512)]          # matmul N<=512 splits of 785
TOKC = [(i * 128, min(128, N - i * 128)) for i in range((N + 127) // 128)]

F32 = mybir.dt.float32
BF = mybir.dt.bfloat16
ADD = mybir.AluOpType.add
MULT = mybir.AluOpType.mult

TRACE = False          # set by test harness for profiled runs
LAST_EXEC_NS = None


def _pbcast(nc, dpool, src, dst, which):
    """Broadcast src [1, F] to dst [P, F] across partitions by bouncing
    through DRAM (DRAM source APs allow a zero partition step; SBUF ones
    don't, and this walrus build can't codegen gpsimd partition_broadcast)."""
    f = src.shape[-1]
    dtmp = dpool.tile([1, f], src.dtype, name=f"dtmp{which}", tag="dtmp",
                      bufs=4)
    nc.gpsimd.dma_start(out=dtmp, in_=src)
    nc.gpsimd.dma_start(out=dst, in_=bass.AP(
        tensor=dtmp.tensor, offset=dtmp.offset,
        ap=[[0, dst.shape[0]]] + [list(dtmp.ap[-1])]))


def _layernorm(nc, pools, wts, xc, which):
    """y = (xc - mu) * rstd in bf16, channel-major. LN affine is folded into
    the consuming matmul weights on the host."""
    wpool, abuf, inbuf, scratch, ppool, pspool, dpool = pools
    s1 = pspool.tile([1, N], F32, name=f"s1{which}", tag="ps")
    s2 = pspool.tile([1, N], F32, name=f"s2{which}", tag="ps")
    for c in range(CC):
        xb = scratch.tile([128, N], BF, name=f"xb{which}{c}", tag="scratch_bf")
        nc.vector.tensor_copy(out=xb, in_=xc[c])
        sq = scratch.tile([128, N], BF, name=f"sq{which}{c}", tag="scratch_bf")
        nc.vector.tensor_mul(out=sq, in0=xc[c], in1=xc[c])
        for (o, w) in NSPLIT:
            nc.tensor.matmul(s1[:, o:o + w], wts['ones_col'], xb[:, o:o + w],
                             start=(c == 0), stop=(c == CC - 1))
            nc.tensor.matmul(s2[:, o:o + w], wts['ones_col'], sq[:, o:o + w],
                             start=(c == 0), stop=(c == CC - 1))
    # var*C = s2 - s1^2/C ; std = sqrt(var*C/C + eps); rstd = 1/std
    t1 = scratch.tile([1, N], F32, name=f"t1{which}", tag="ln_small")
    nc.scalar.square(out=t1, in_=s1)
    v384 = scratch.tile([1, N], F32, name=f"v384{which}", tag="ln_small")
    nc.vector.scalar_tensor_tensor(out=v384, in0=t1, scalar=-1.0 / C, in1=s2,
                                   op0=MULT, op1=ADD)
    std = scratch.tile([1, N], F32, name=f"std{which}", tag="ln_small")
    nc.scalar.activation(out=std, in_=v384,
                         func=mybir.ActivationFunctionType.Sqrt,
                         bias=1e-5, scale=1.0 / C)
    rstd = scratch.tile([1, N], BF, name=f"rstd{which}", tag="ln_small")
    with nc.allow_low_precision(reason="rstd feeds bf16 y anyway"):
        nc.vector.reciprocal(out=rstd, in_=std)
    nm = scratch.tile([1, N], BF, name=f"nm{which}", tag="ln_small")
    nc.vector.scalar_tensor_tensor(out=nm, in0=s1, scalar=-1.0 / C, in1=rstd,
                                   op0=MULT, op1=MULT)
    # broadcast rstd and nm across partitions via K=1 ones-outer matmuls
    # (keeps the LN critical path on PE; avoids DMA-ring round trips)
    rbp = pspool.tile([128, N], F32, name=f"rbp{which}", tag="ps")
    nbp = pspool.tile([128, N], F32, name=f"nbp{which}", tag="ps")
    for (o, w) in NSPLIT:
        nc.tensor.matmul(rbp[:, o:o + w], wts['ones_row'], rstd[:, o:o + w],
                         start=True, stop=True)
        nc.tensor.matmul(nbp[:, o:o + w], wts['ones_row'], nm[:, o:o + w],
                         start=True, stop=True)
    y = []
    for c in range(CC):
        yt = scratch.tile([128, N], BF, name=f"yt{which}{c}", tag="scratch_bf")
        nc.vector.tensor_mul(out=yt, in0=xc[c], in1=rbp)
        yb = abuf.tile([128, N], BF, name=f"y{which}{c}", tag=f"y{c}", bufs=2)
        nc.vector.tensor_add(out=yb, in0=yt, in1=nbp)
        y.append(yb)
    return y


def _load_inputs(nc, pools, prm, b):
    wpool, abuf, inbuf, scratch, ppool, pspool, dpool = pools
    xpad, xf32, xcs = [], [], []
    for c in range(CC):
        xpad_t = inbuf.tile([128, PADW, PADW], BF, name=f"xpad{c}",
                            tag=f"xpad{c}")
        nc.sync.dma_start(out=xpad_t,
                          in_=prm['xpad_bf'][b, c * 128:(c + 1) * 128]
                          .rearrange("p (h w) -> p h w", h=PADW))
        xpad.append(xpad_t)
    for c in range(CC):
        xf_t = inbuf.tile([128, HWP], F32, name=f"xf{c}", tag=f"xf{c}")
        nc.sync.dma_start(out=xf_t, in_=prm['x_f32'][b, c * 128:(c + 1) * 128])
        xf32.append(xf_t)
    for c in range(CC):
        xc_t = abuf.tile([128, N], F32, name=f"xc{c}", tag=f"xc{c}",
                         bufs=2)
        nc.sync.dma_start(out=xc_t[:, 0:1],
                          in_=prm['cls'][b, c * 128:(c + 1) * 128])
        xcs.append(xc_t)
    return xpad, xf32, xcs


def _emit_batch(nc, tc, pools, wts, prm, b, preloaded):
    wpool, abuf, inbuf, scratch, ppool, pspool, dpool = pools

    # ---------------- Phase A: conv + residual + xc^T assembly ------------
    xpad, xf32, xc = preloaded
    for c in range(CC):
        xc_t = xc[c]
        for half in range(2):
            cps = pspool.tile([128, 392], F32, name=f"cps{c}{half}", tag="ps")
            r0 = half * 14
            for t in range(9):
                ti, tj = divmod(t, 3)
                rhs = xpad[c][:, ti + r0: ti + r0 + 14, tj: tj + W]
                nc.tensor.matmul(cps, wts['dconv'][c][:, t, :], rhs,
                                 start=(t == 0), stop=(t == 8))
            nc.vector.scalar_tensor_tensor(
                out=xc_t[:, 1 + half * 392: 1 + (half + 1) * 392], in0=cps,
                scalar=wts['conv_b'][:, c:c + 1],
                in1=xf32[c][:, half * 392:(half + 1) * 392], op0=ADD, op1=ADD)

    # ---------------- Phase B: LN1 -> y ----------------------------------
    y = _layernorm(nc, pools, wts, xc, f"a{b}")

    # ---------------- Phase C: q^T, k^T (head-padded), v (token-major) ---
    qT, kT = [], []
    for name, wkey, bkey, dst in (("q", 'wq', 'bq', qT), ("k", 'wk', 'bk', kT)):
        for j in range(QROWS // 128):
            ps = pspool.tile([128, N], F32, name=f"ps{name}{j}", tag="ps")
            for c in range(CC):
                for (o, w) in NSPLIT:
                    nc.tensor.matmul(ps[:, o:o + w],
                                     wts[wkey][c][:, j * 128:(j + 1) * 128],
                                     y[c][:, o:o + w],
                                     start=(c == 0), stop=(c == CC - 1))
            ot = abuf.tile([128, N], BF, name=f"{name}T{j}", tag=f"{name}T{j}")
            nc.vector.tensor_scalar_add(out=ot, in0=ps,
                                        scalar1=wts[bkey][:, j:j + 1])
            dst.append(ot)

    v_sb = []
    for mc, (ms, mr) in enumerate(TOKC):
        ps = pspool.tile([128, C], F32, name=f"psv{mc}", tag="ps")
        for c in range(CC):
            nc.tensor.matmul(ps[:mr, :], y[c][:, ms:ms + mr], wts['wv'][c],
                             start=(c == 0), stop=(c == CC - 1))
        vt = abuf.tile([128, HEADS, D + 1], BF, name=f"v{mc}", tag=f"v{mc}")
        nc.gpsimd.memset(vt[:mr, :, 0:1], 1.0)
        nc.vector.scalar_tensor_tensor(
            out=vt[:mr, :, 1:D + 1],
            in0=ps[:mr, :].rearrange("p (h d) -> p h d", h=HEADS),
            scalar=1.0,
            in1=wts['bv_bcast'][:mr, :].rearrange("p (h d) -> p h d", h=HEADS),
            op0=MULT, op1=ADD)
        v_sb.append(vt)

    # ---------------- Phase D: attention, head at a time ------------------
    o_bf = []
    r0_row = abuf.tile([1, HEADS], BF, name="r0row", tag="r0row")
    gcols = [abuf.tile([128, HEADS], BF, name=f"gcol{mc}", tag=f"gcol{mc}")
             for mc in range(len(TOKC))]

    for h in range(HEADS):
        jt, jr = h // 2, QP * (h % 2)
        ops = pspool.tile([D + 1, N], F32, name=f"ops{h}", tag="ps")
        for mc, (ms, mr) in enumerate(TOKC):
            sps = pspool.tile([128, N], F32, name=f"sps{h}{mc}", tag="ps")
            for (o, w) in NSPLIT:
                nc.tensor.matmul(sps[:mr, o:o + w],
                                 kT[jt][jr:jr + QP, ms:ms + mr],
                                 qT[jt][jr:jr + QP, o:o + w],
                                 start=True, stop=True)
            pt = ppool.tile([128, N], BF, name=f"p{h}{mc}", tag="pt")
            nc.scalar.activation(out=pt[:mr, :], in_=sps[:mr, :],
                                 func=mybir.ActivationFunctionType.Exp,
                                 scale=SCALE)
            nc.gpsimd.tensor_copy(out=gcols[mc][:mr, h:h + 1],
                                  in_=pt[:mr, 0:1])
            for (o, w) in NSPLIT:
                nc.tensor.matmul(ops[:, o:o + w], v_sb[mc][:mr, h, :],
                                 pt[:mr, o:o + w],
                                 start=(mc == 0), stop=(mc == len(TOKC) - 1))
        rec = abuf.tile([1, N], BF, name=f"rec{h}", tag="rec", bufs=2)
        with nc.allow_low_precision(reason="softmax recip consumed in bf16"):
            nc.vector.reciprocal(out=rec, in_=ops[0:1, :])
        nc.gpsimd.tensor_copy(out=r0_row[:, h:h + 1], in_=rec[:, 0:1])
        rbs = scratch.tile([D + 1, N], BF, name=f"rbs{h}", tag="rbs", bufs=2)
        _pbcast(nc, pools[-1], rec, rbs, f"rec{h}")
        ob = abuf.tile([D + 1, N], BF, name=f"ob{h}", tag=f"ob{h}")
        nc.vector.tensor_mul(out=ob, in0=ops, in1=rbs)
        o_bf.append(ob)

    # ---------------- global_attn ----------------------------------------
    r0b = pspool.tile([128, HEADS], F32, name="r0b", tag="ps")
    nc.tensor.matmul(r0b, wts['ones_row'], r0_row, start=True, stop=True)
    for mc, (ms, mr) in enumerate(TOKC):
        gs = abuf.tile([128, 1], F32, name=f"gs{mc}", tag=f"gs{mc}")
        nc.vector.scalar_tensor_tensor(
            out=gcols[mc][:mr, :], in0=gcols[mc][:mr, :], scalar=1.0 / HEADS,
            in1=r0b[:mr, :], op0=MULT, op1=MULT, accum_out=gs[:mr, :])
        if mc == 0:
            nc.gpsimd.dma_start(out=prm['gattn'][b, 0:mr - 1], in_=gs[1:mr, :])
        else:
            nc.gpsimd.dma_start(out=prm['gattn'][b, ms - 1:ms - 1 + mr],
                                in_=gs[:mr, :])

    # ---------------- Phase E: proj + residual ---------------------------
    for c in range(CC):
        ps = pspool.tile([128, N], F32, name=f"psp{c}", tag="ps")
        for hh in range(HEADS):
            for (o, w) in NSPLIT:
                nc.tensor.matmul(ps[:, o:o + w],
                                 wts['wproj'][hh][:, c * 128:(c + 1) * 128],
                                 o_bf[hh][:, o:o + w],
                                 start=(hh == 0), stop=(hh == HEADS - 1))
        nc.vector.scalar_tensor_tensor(
            out=xc[c], in0=ps, scalar=wts['bproj'][:, c:c + 1],
            in1=xc[c], op0=ADD, op1=ADD)

    # ---------------- Phase F/G: LN2 + MLP -------------------------------
    y2 = _layernorm(nc, pools, wts, xc, f"b{b}")
    h_bf = []
    for j in range(HC):
        ps = pspool.tile([128, N], F32, name=f"psh{j}", tag="ps")
        for c in range(CC):
            for (o, w) in NSPLIT:
                nc.tensor.matmul(ps[:, o:o + w],
                                 wts['wfc1'][c][:, j * 128:(j + 1) * 128],
                                 y2[c][:, o:o + w],
                                 start=(c == 0), stop=(c == CC - 1))
        ht = abuf.tile([128, N], BF, name=f"h{j}", tag=f"h{j}")
        nc.scalar.activation(out=ht, in_=ps,
                             func=mybir.ActivationFunctionType.Gelu,
                             bias=wts['bfc1'][:, j:j + 1], scale=1.0)
        h_bf.append(ht)
    for c in range(CC):
        ps = pspool.tile([128, N], F32, name=f"psf{c}", tag="ps")
        for j in range(HC):
            for (o, w) in NSPLIT:
                nc.tensor.matmul(ps[:, o:o + w],
                                 wts['wfc2'][j][:, c * 128:(c + 1) * 128],
                                 h_bf[j][:, o:o + w],
                                 start=(j == 0), stop=(j == HC - 1))
        nc.vector.scalar_tensor_tensor(
            out=xc[c], in0=ps, scalar=wts['bfc2'][:, c:c + 1],
            in1=xc[c], op0=ADD, op1=ADD)

    # ---------------- Phase H: outputs -----------------------------------
    for c in range(CC):
        nc.gpsimd.dma_start(out=prm['x_out'][b, c * 128:(c + 1) * 128, :],
                            in_=xc[c][:, 1:N])
        nc.gpsimd.dma_start(out=prm['cls_out'][b, c * 128:(c + 1) * 128],
                            in_=xc[c][:, 0:1])


def _register_const(nc, dtype, value):
    t = nc.alloc_sbuf_tensor(f"const-{dtype.name}-{value}", [128, 1], dtype)
    nc.gpsimd.memset(t.ap(), value)
    nc.const_aps.aps[(dtype, value)] = t.ap()


def build_program():
    nc = bass.Bass()
    _register_const(nc, mybir.dt.float32, 1e-5)
    nc.all_engine_barrier()
    prm = {}
    dp = nc.declare_dram_parameter
    prm['xpad_bf'] = dp("xpad_bf", [BPC, C, PADW * PADW], BF, isOutput=False)
    prm['x_f32'] = dp("x_f32", [BPC, C, HWP], F32, isOutput=False)
    prm['cls'] = dp("cls", [BPC, C, 1], F32, isOutput=False)
    prm['dconv'] = dp("dconv", [CC, 128, 9, 128], BF, isOutput=False)
    prm['conv_b'] = dp("conv_b", [128, CC], F32, isOutput=False)
    prm['wq'] = dp("wq", [C, QROWS], BF, isOutput=False)
    prm['wk'] = dp("wk", [C, QROWS], BF, isOutput=False)
    prm['wv'] = dp("wv", [C, C], BF, isOutput=False)
    prm['bq'] = dp("bq", [128, QROWS // 128], F32, isOutput=False)
    prm['bk'] = dp("bk", [128, QROWS // 128], F32, isOutput=False)
    prm['bv_bcast'] = dp("bv_bcast", [128, C], F32, isOutput=False)
    prm['wproj'] = dp("wproj", [HEADS, D + 1, C], BF, isOutput=False)
    prm['bproj'] = dp("bproj", [128, CC], F32, isOutput=False)
    prm['wfc1'] = dp("wfc1", [C, HID], BF, isOutput=False)
    prm['bfc1'] = dp("bfc1", [128, HC], F32, isOutput=False)
    prm['wfc2'] = dp("wfc2", [HID, C], BF, isOutput=False)
    prm['bfc2'] = dp("bfc2", [128, CC], F32, isOutput=False)
    prm['cls_out'] = dp("cls_out", [BPC, C, 1], F32, isOutput=True)
    prm['x_out'] = dp("x_out", [BPC, C, HWP], F32, isOutput=True)
    prm['gattn'] = dp("gattn", [BPC, HWP], F32, isOutput=True)

    with tile.TileContext(nc) as tc:
        import contextlib
        with contextlib.ExitStack() as ctx:
            wpool = ctx.enter_context(tc.tile_pool(name="wpool", bufs=1))
            abuf = ctx.enter_context(tc.tile_pool(name="abuf", bufs=1))
            inbuf = ctx.enter_context(tc.tile_pool(name="inbuf", bufs=2))
            scratch = ctx.enter_context(tc.tile_pool(name="scratch", bufs=3))
            ppool = ctx.enter_context(tc.tile_pool(name="ppool", bufs=3))
            pspool = ctx.enter_context(
                tc.tile_pool(name="pspool", bufs=4, space="PSUM"))
            dpool = ctx.enter_context(
                tc.tile_pool(name="dpool", bufs=1, space="DRAM"))
            pools = (wpool, abuf, inbuf, scratch, ppool, pspool, dpool)

            wts = {}
            dconv = []
            for c in range(CC):
                t = wpool.tile([128, 9, 128], BF, name=f"dconv{c}",
                               tag=f"dconv{c}")
                nc.sync.dma_start(out=t, in_=prm['dconv'][c])
                dconv.append(t)
            wts['dconv'] = dconv
            for key, rows, width in (('wq', C, QROWS), ('wk', C, QROWS),
                                     ('wv', C, C)):
                ts = []
                for c in range(rows // 128):
                    t = wpool.tile([128, width], BF, name=f"{key}{c}",
                                   tag=f"{key}{c}")
                    nc.sync.dma_start(out=t,
                                      in_=prm[key][c * 128:(c + 1) * 128])
                    ts.append(t)
                wts[key] = ts
            for key, shape in (('conv_b', [128, CC]), ('bq', [128, 4]),
                               ('bk', [128, 4]), ('bv_bcast', [128, C])):
                t = wpool.tile(shape, F32, name=f"w_{key}", tag=f"w_{key}")
                nc.sync.dma_start(out=t, in_=prm[key][:, :])
                wts[key] = t
            ones_col = wpool.tile([128, 1], BF, name="ones_col",
                                  tag="ones_col")
            nc.vector.memset(ones_col, 1.0)
            wts['ones_col'] = ones_col
            ones_row = wpool.tile([1, 128], BF, name="ones_row",
                                  tag="ones_row")
            nc.vector.memset(ones_row, 1.0)
            wts['ones_row'] = ones_row

            # batch-0 inputs ahead of the late-needed weights so conv/LN1
            # start ~30us earlier
            pre0 = _load_inputs(nc, pools, prm, 0)

            ts = []
            for c in range(CC):
                t = wpool.tile([128, HID], BF, name=f"wfc1{c}",
                               tag=f"wfc1{c}")
                nc.sync.dma_start(out=t, in_=prm['wfc1'][c * 128:(c + 1) * 128])
                ts.append(t)
            wts['wfc1'] = ts
            ts = []
            for j in range(HC):
                t = wpool.tile([128, C], BF, name=f"wfc2{j}", tag=f"wfc2{j}")
                nc.sync.dma_start(out=t, in_=prm['wfc2'][j * 128:(j + 1) * 128])
                ts.append(t)
            wts['wfc2'] = ts
            ts = []
            for hh in range(HEADS):
                t = wpool.tile([D + 1, C], BF, name=f"wproj{hh}",
                               tag=f"wproj{hh}")
                nc.sync.dma_start(out=t, in_=prm['wproj'][hh])
                ts.append(t)
            wts['wproj'] = ts
            for key, shape in (('bproj', [128, CC]), ('bfc1', [128, HC]),
                               ('bfc2', [128, CC])):
                t = wpool.tile(shape, F32, name=f"w_{key}", tag=f"w_{key}")
                nc.sync.dma_start(out=t, in_=prm[key][:, :])
                wts[key] = t

            pre1 = _load_inputs(nc, pools, prm, 1)
            for b, pre in ((0, pre0), (1, pre1)):
                _emit_batch(nc, tc, pools, wts, prm, b, pre)
    _split_excess_waits(nc)
    return nc, prm


_CACHED = None


def _get_program():
    global _CACHED
    if _CACHED is None:
        _CACHED = build_program()
    return _CACHED


def _prep_host(inputs):
    """Fold LN affine into weights, transpose/pad into lhsT layouts, pad x
    for SAME conv, cast matmul operands to bf16."""
    f32 = np.float32
    x = np.asarray(inputs['x'], f32)
    cls_token = np.asarray(inputs['cls_token'], f32)
    conv_w = np.asarray(inputs['conv_w'], f32)
    conv_b = np.asarray(inputs['conv_b'], f32)
    ln1_g = np.asarray(inputs['ln1_g'], f32)
    ln1_b = np.asarray(inputs['ln1_b'], f32)
    qkv_w = np.asarray(inputs['qkv_w'], f32)
    proj_w = np.asarray(inputs['proj_w'], f32)
    proj_b = np.asarray(inputs['proj_b'], f32)
    ln2_g = np.asarray(inputs['ln2_g'], f32)
    ln2_b = np.asarray(inputs['ln2_b'], f32)
    fc1_w = np.asarray(inputs['fc1_w'], f32)
    fc1_b = np.asarray(inputs['fc1_b'], f32)
    fc2_w = np.asarray(inputs['fc2_w'], f32)
    fc2_b = np.asarray(inputs['fc2_b'], f32)

    def colmajor(v):          # [C*] vector -> [128, C*/128] chunk-col layout
        return np.ascontiguousarray(v.reshape(-1, 128).T).astype(f32)

    shared = {}
    wt = conv_w[:, 0].reshape(C, 9)
    dconv = np.zeros((CC, 128, 9, 128), f32)
    idx = np.arange(128)
    for c in range(CC):
        dconv[c, idx, :, idx] = wt[c * 128:(c + 1) * 128, :]
    shared['dconv'] = dconv.astype(BF16)
    shared['conv_b'] = colmajor(conv_b)

    qkv_g = qkv_w * ln1_g[None, :]
    qkv_bias = qkv_w @ ln1_b
    wq_f, wk_f, wv_f = qkv_g[0:C], qkv_g[C:2 * C], qkv_g[2 * C:3 * C]
    bq_f, bk_f, bv_f = qkv_bias[0:C], qkv_bias[C:2 * C], qkv_bias[2 * C:3 * C]

    def pad_qk(wf):
        out = np.zeros((C, QROWS), f32)
        for hh in range(HEADS):
            out[:, hh * QP: hh * QP + D] = wf[hh * D:(hh + 1) * D, :].T
        return out

    def pad_qk_bias(bf_):
        out = np.zeros((QROWS,), f32)
        for hh in range(HEADS):
            out[hh * QP: hh * QP + D] = bf_[hh * D:(hh + 1) * D]
        return colmajor(out)

    shared['wq'] = pad_qk(wq_f).astype(BF16)
    shared['wk'] = pad_qk(wk_f).astype(BF16)
    shared['bq'] = pad_qk_bias(bq_f)
    shared['bk'] = pad_qk_bias(bk_f)
    shared['wv'] = wv_f.T.copy().astype(BF16)
    shared['bv_bcast'] = np.tile(bv_f[None, :], (128, 1)).astype(f32)
    wproj = np.zeros((HEADS, D + 1, C), f32)
    wproj[:, 1:, :] = proj_w.T.reshape(HEADS, D, C)
    shared['wproj'] = wproj.astype(BF16)
    shared['bproj'] = colmajor(proj_b)
    shared['wfc1'] = (fc1_w * ln2_g[None, :]).T.copy().astype(BF16)
    shared['bfc1'] = colmajor(fc1_b + fc1_w @ ln2_b)
    shared['wfc2'] = fc2_w.T.copy().astype(BF16)
    shared['bfc2'] = colmajor(fc2_b)

    xf = x.reshape(B, C, HWP)
    xpad = np.zeros((B, C, PADW, PADW), f32)
    xpad[:, :, 1:1 + H, 1:1 + W] = x
    xpad_bf = xpad.reshape(B, C, PADW * PADW).astype(BF16)
    clsr = np.ascontiguousarray(cls_token[:, 0, :]).reshape(B, C, 1)

    in_maps = []
    for core in range(NCORES):
        sl = slice(core * BPC, (core + 1) * BPC)
        m = dict(shared)
        m['xpad_bf'] = np.ascontiguousarray(xpad_bf[sl])
        m['x_f32'] = np.ascontiguousarray(xf[sl])
        m['cls'] = np.ascontiguousarray(clsr[sl])
        in_maps.append(m)
    return in_maps


def kernel(**inputs):
    global LAST_EXEC_NS
    from concourse.bass_utils import run_bass_kernel_spmd
    nc, prm = _get_program()
    in_maps = _prep_host(inputs)
    res = run_bass_kernel_spmd(nc, in_maps, core_ids=list(range(NCORES)),
                               trace=TRACE)
    LAST_EXEC_NS = res.exec_time_ns
    cls_out = np.zeros((B, 1, C), np.float32)
    x_out = np.zeros((B, C, H, W), np.float32)
    gattn = np.zeros((B, HWP), np.float32)
    for core in range(NCORES):
        r = res.results[core]
        sl = slice(core * BPC, (core + 1) * BPC)
        cls_out[sl, 0, :] = r['cls_out'][:, :, 0]
        x_out[sl] = r['x_out'].reshape(BPC, C, H, W)
        gattn[sl] = r['gattn']
    return cls_out, x_out, gattn
